# revision 27
# baseline (speedup 1.0000x reference)
"""CARAFE forward as a Bass/Tile kernel for 8 Trainium2 NeuronCores.

Problem (per sample, B=16 total, data-parallel 2 samples/core):
  x [4096, 256] -> down 1x1 conv (256->64) -> enc 3x3 conv (64->36)
  -> pixel_shuffle + softmax over 9 patch weights per upsampled pixel
  -> content-adaptive reassembly of out-conv features -> out [16384, 256]

Key algebraic fusion: the trailing 1x1 conv (out_w) commutes with the
reassembly, so we conv first on the 4096-pixel grid (v = x @ wo), then
reassemble v directly into the final output: 4x fewer conv FLOPs.

Reassembly mapping (one 105-partition matmul per output row per w-half):
the contraction dim packs 3 v-rows x 35 w-columns of vb = v + bo, where
columns/rows outside the image hold bo itself -- softmax weights sum to
1, so reassembling vb with bo padding yields reassemble(v) + bo exactly,
borders included, and the out-conv bias needs no separate add. Two
overlapping w-half tiles cover the 3x3 w-taps without cross-tile halo.
The banded stationary M [112, 256] is built per row by one gpsimd
local_scatter from a replicated-kt tile (ktrep) that 36 small
partition-shifted SBUF DMAs assemble from the softmaxed kt; v3 tiles
are assembled from vA by ~26 batched partition-remapping SBUF DMAs.

Emission order runs both samples' front-ends before either reassembly so
the PE never stalls on the softmax/scatter dependency chain.
"""
import os
import sys
import numpy as np

sys.path.insert(0, "/opt/trn_rl_repo")

import concourse.bass as bass
import concourse.mybir as mybir
import concourse.tile as tile
from concourse.bass_utils import run_bass_kernel_spmd

F32 = mybir.dt.float32
F16 = mybir.dt.float16
I16 = mybir.dt.int16

U, K, C, H, W = 2, 3, 256, 64, 64
HW = H * W                      # 4096
C4 = C // 4                     # 64
NK, NU, NCH = 9, 4, 36
NCORES = 8
BPC = 2                         # samples per core

PH = 16                         # v3 piece rows (4 pieces per sample)
NQ = 105                        # live v3 partitions: 3 slots x 35 w (incl bo pad)
NIDX = 24                       # scatter idxs per partition: 2 halves x 3 j x 4 u

_cache = {}
last_result = None


# ---------------------------------------------------------------------------
# host-side constant tables
# ---------------------------------------------------------------------------

def _build_idx_tables():
    """[112, 24] int16 scatter indices (single variant).

    Partition q = s*35 + wt (s = v-row slot = tap i, wt = tile w col;
    wt column holds v value at w_v = wt + WOFF, WOFF = -1 / +30, with
    bo padding outside the image so softmax weight-sum-1 supplies the
    out-conv bias). Entry jj = half*12 + j*4 + u scatters ktrep value
    kt[h, w, 3s+j, u] (w = wt-j / wt+31-j) to M col
    half*128 + 64*(u//2) + 2*d + u%2, d = w - 32*half; -1 when the out
    pixel w falls outside the half.
    """
    idx = np.full((112, NIDX), -1, np.int16)
    for q in range(NQ):
        s, wt = q // 35, q % 35
        for half in (0, 1):
            for j in range(3):
                w = wt - j if half == 0 else wt + 31 - j
                if not 32 * half <= w < 32 * half + 32:
                    continue
                d = w - 32 * half
                for u in range(NU):
                    idx[q, half * 12 + j * 4 + u] = \
                        half * 128 + 64 * (u // 2) + 2 * d + (u % 2)
    return idx


# ---------------------------------------------------------------------------
# device program
# ---------------------------------------------------------------------------

def _build_program():
    nc = bass.Bass()

    x2 = nc.declare_dram_parameter("x2", [BPC, HW, C], F16, isOutput=False)
    wd = nc.declare_dram_parameter("wd", [128, 128], F16, isOutput=False)
    bd = nc.declare_dram_parameter("bd", [C4, 1], F32, isOutput=False)
    weA = nc.declare_dram_parameter("weA", [128, 108], F16, isOutput=False)
    weB = nc.declare_dram_parameter("weB", [C4, 108], F16, isOutput=False)
    be = nc.declare_dram_parameter("be", [NCH, 1], F32, isOutput=False)
    wo = nc.declare_dram_parameter("wo", [128, 512], F16, isOutput=False)
    bo16 = nc.declare_dram_parameter("bo16", [128, C], F16, isOutput=False)
    bofill = nc.declare_dram_parameter("bofill", [35, 16 * 2 * C], F16,
                                       isOutput=False)
    out = nc.declare_dram_parameter("out", [BPC, 4 * HW, C], F16, True)

    idx_dram = nc.inline_tensor(_build_idx_tables(), name="idxtab")
    ident_dram = nc.inline_tensor(np.eye(128, dtype=np.float32), name="ident")

    with tile.TileContext(nc) as tc:
        _emit(tc, nc, x2, wd, bd, weA, weB, be, wo, bo16, bofill, out,
              idx_dram, ident_dram)
    # raw-Bass path skips Bacc's extended-inst codegen; without this the
    # NEFF compiler sees empty .instr bytes -> "ISA wrong length"
    from concourse.library_overlay import lower_extended_insts
    lower_extended_insts(nc)
    _split_excess_waits(nc)
    return nc


def _split_excess_waits(nc, cap=1):
    """Each TPB instruction has a single EVENTS wait slot; walrus rejects
    multi-wait instructions ("Too many sync wait commands"). Move excess
    waits onto same-engine NoOps immediately before the instruction —
    semantically identical since the engine blocks at the same PC."""
    nid = [0]
    for f in nc.m.functions:
        for b in f.blocks:
            insts = b.instructions
            i = 0
            while i < len(insts):
                ins = insts[i]
                si = getattr(ins, 'sync_info', None)
                if si is not None and si.on_wait and len(si.on_wait) > cap:
                    waits = list(si.on_wait)
                    for w in waits[:-cap]:
                        nop = mybir.InstNoOp(name=f"nopw-{nid[0]}", ins=[],
                                             outs=[])
                        nid[0] += 1
                        nop.engine = ins.engine
                        nop.sync_info = mybir.SyncInfo(on_wait=[w],
                                                       on_update=[])
                        insts.insert(i, nop)
                        i += 1
                    ins.sync_info = mybir.SyncInfo(
                        on_wait=waits[-cap:],
                        on_update=list(si.on_update or []))
                i += 1


def _emit(tc, nc, x2, wd, bd, weA, weB, be, wo, bo16, bofill, out,
          idx_dram, ident_dram):
    from contextlib import ExitStack
    ablate = set(os.environ.get("KABLATE", "").split(","))
    ctx = ExitStack()
    with ctx:
        consts = ctx.enter_context(tc.tile_pool(name="consts", bufs=1))
        xt_pool = ctx.enter_context(tc.tile_pool(name="xt", bufs=1))
        o1_pool = ctx.enter_context(tc.tile_pool(name="o1", bufs=1))
        enc_pool = ctx.enter_context(tc.tile_pool(name="enc", bufs=1))
        kt_pool = ctx.enter_context(tc.tile_pool(name="kt", bufs=1))
        kr_pool = ctx.enter_context(tc.tile_pool(name="kr", bufs=2))
        v_pool = ctx.enter_context(tc.tile_pool(name="v", bufs=2))
        v3_pool = ctx.enter_context(tc.tile_pool(name="v3", bufs=6))
        mkt_pool = ctx.enter_context(tc.tile_pool(name="mkt", bufs=6))
        out_pool = ctx.enter_context(tc.tile_pool(name="ob", bufs=6))
        ps_dek = ctx.enter_context(tc.tile_pool(name="psdek", bufs=2, space="PSUM"))
        ps_v = ctx.enter_context(tc.tile_pool(name="psv", bufs=2, space="PSUM"))
        ps_o = ctx.enter_context(tc.tile_pool(name="pso", bufs=4, space="PSUM"))

        # ---- constants to SBUF ----
        wd_sb = consts.tile([128, 128], F16)
        nc.sync.dma_start(wd_sb[:], wd[:])
        bd_sb = consts.tile([C4, 1], F32)
        nc.sync.dma_start(bd_sb[:], bd[:])
        weA_sb = consts.tile([128, 108], F16)
        nc.sync.dma_start(weA_sb[:], weA[:])
        weB_sb = consts.tile([C4, 108], F16)
        nc.sync.dma_start(weB_sb[:], weB[:])
        be_sb = consts.tile([NCH, 1], F32)
        nc.sync.dma_start(be_sb[:], be[:])
        wo_sb = consts.tile([128, 512], F16)
        nc.sync.dma_start(wo_sb[:], wo[:])
        idx_sb = consts.tile([112, NIDX], I16)
        nc.sync.dma_start(idx_sb[:], idx_dram[:])
        bo16_sb = consts.tile([128, C], F16)
        nc.sync.dma_start(bo16_sb[:], bo16[:])
        id_sb = consts.tile([128, 128], F32)
        nc.sync.dma_start(id_sb[:], ident_dram[:])

        from concourse import library_config
        nc.gpsimd.load_library(library_config.local_scatter)

        nrep = int(os.environ.get("KREPEAT", "1"))
        xts, vAs, v3s, kreps = {}, {}, {}, {}

        def front(s):
            # ---- xT [256 c, 4096 pos] via DMA XBAR transpose from DRAM ----
            xt0 = xt_pool.tile([128, HW], F16, tag="xt0")
            xt1 = xt_pool.tile([128, HW], F16, tag="xt1")
            xts[s] = (xt0, xt1)
            # NOTE: both on ONE engine: concurrent XBAR transposes from two
            # engines garble each other (shared xbar state); same-engine
            # back-to-back is safe. Keep un-chunked: a row-sliced src
            # mis-lowers (emitted src pattern spans the full tensor).
            nc.sync.dma_start_transpose(xt0[:], x2[s, :, 0:128])
            nc.sync.dma_start_transpose(xt1[:], x2[s, :, 128:256])

            # ---- out1d: padded + row-shift-duplicated down-conv output ----
            o1 = o1_pool.tile([128, 66 * 66], F16, tag="o1")
            o1v = o1[:].rearrange("p (r c) -> p r c", r=66)
            nc.vector.memset(o1v[:, 0:1, :], 0.0)
            nc.vector.memset(o1v[:, 65:66, :], 0.0)
            nc.vector.memset(o1v[:, :, 0:1], 0.0)
            nc.vector.memset(o1v[:, :, 65:66], 0.0)

            for n in range(8):          # 8 h-rows per 512-chunk
                pd = ps_dek.tile([C4, 512], F32, tag="psdek")
                nc.tensor.matmul(pd[:], wd_sb[:, 0:64],
                                 xt0[:, n * 512:(n + 1) * 512],
                                 start=True, stop=False)
                nc.tensor.matmul(pd[:], wd_sb[:, 64:128],
                                 xt1[:, n * 512:(n + 1) * 512],
                                 start=False, stop=True)
                pdv = pd[:].rearrange("p (r c) -> p r c", r=8)
                nc.vector.tensor_scalar_add(
                    o1v[0:64, 1 + n * 8:9 + n * 8, 1:65], pdv, bd_sb[:])
                nc.vector.tensor_scalar_add(
                    o1v[64:128, n * 8:8 + n * 8, 1:65], pdv, bd_sb[:])

            # ---- enc conv -> enc_out [36, 4096] ----
            enc_sb = enc_pool.tile([NCH, HW], F32, tag="enc")
            for n in range(8):
                pe = ps_dek.tile([NCH, 512], F32, tag="psdek")
                first = True
                for dj in range(3):
                    nc.tensor.matmul(
                        pe[:], weA_sb[:, dj * 36:(dj + 1) * 36],
                        o1v[:, n * 8:n * 8 + 8, dj:dj + 64],
                        start=first, stop=False)
                    first = False
                    nc.tensor.matmul(
                        pe[:], weB_sb[:, dj * 36:(dj + 1) * 36],
                        o1v[0:64, n * 8 + 2:n * 8 + 10, dj:dj + 64],
                        start=False, stop=(dj == 2))
                nc.vector.tensor_scalar_add(
                    enc_sb[:, n * 512:(n + 1) * 512], pe[:], be_sb[:])

            # ---- kt transpose: [4096 pos, 36] chunks + softmax ----
            kts = kt_pool.tile([128, 32 * NCH], F32, tag="kts")
            for c in range(32):
                pt = ps_dek.tile([128, NCH], F32, tag="psdek")
                nc.tensor.matmul(pt[:], enc_sb[:, c * 128:(c + 1) * 128],
                                 id_sb[0:NCH, 0:NCH], is_transpose=True)
                nc.vector.tensor_copy(kts[:, c * NCH:(c + 1) * NCH], pt[:])

            ea = kt_pool.tile([128, 32 * NCH], F32, tag="ea")
            nc.scalar.activation(ea[:], kts[:],
                                 mybir.ActivationFunctionType.Exp)
            sums = kt_pool.tile([128, 128], F32, tag="sums")
            nc.vector.reduce_sum(sums[:].rearrange("p (c u) -> p c u", u=NU),
                                 ea[:].rearrange("p (c k u) -> p c u k",
                                                 k=NK, u=NU),
                                 axis=mybir.AxisListType.X)
            rec = kt_pool.tile([128, 128], F32, tag="rec")
            nc.vector.reciprocal(rec[:], sums[:])
            # kt16 [128 pos%128, 32 chunks x (k,u)] fp16 softmaxed weights
            kt16 = kt_pool.tile([128, 32 * NCH], F16, tag="kt16")
            recb = rec[:].rearrange("p (c u) -> p c u", u=NU) \
                         .unsqueeze(2).broadcast_to([128, 32, NK, NU])
            nc.vector.tensor_tensor(
                kt16[:].rearrange("p (c k u) -> p c k u", k=NK, u=NU),
                ea[:].rearrange("p (c k u) -> p c k u", k=NK, u=NU),
                recb, mybir.AluOpType.mult)

            # ---- ktrep [112, 64 h x 24]: per-partition scatter source ----
            # ktrep[s*35+wt, h*24 + hf*12 + j*4 + u] = kt[h, w, 3s+j, u],
            # w = wt - j (hf=0) / wt + 31 - j (hf=1)
            krep = kr_pool.tile([112, 64 * NIDX], F16, tag="krep")
            kreps[s] = krep
            kv = kt16[:].rearrange("(e w) (hp k u) -> e w hp k u",
                                   e=2, k=NK, u=NU)
            dv = krep[:].rearrange("q (hp e hf j u) -> q hp e hf j u",
                                   hp=32, e=2, hf=2, j=3)
            for sl in range(3):
                for hf in range(2):
                    for j in range(3):
                        off = -j if hf == 0 else 31 - j
                        wlo = j if hf == 0 else j + 1
                        kk = 3 * sl + j
                        for e in range(2):
                            eng = (nc.gpsimd, nc.scalar,
                                   nc.sync)[(6 * sl + 2 * hf + j + e) % 3]
                            eng.dma_start(
                                dv[sl * 35 + wlo:sl * 35 + wlo + 32,
                                   :, e, hf, j, :],
                                kv[e, wlo + off:wlo + off + 32, :, kk, :])

            # ---- v = x @ wo (bias via reassembly bias row) ----
            vA = v_pool.tile([128, 32 * C], F16, tag="vA")
            vAs[s] = vA
            for t in range(32):
                pv = ps_v.tile([128, C], F32, tag="psv")
                nc.tensor.matmul(pv[:], xt0[:, t * 128:(t + 1) * 128],
                                 wo_sb[:, 0:256], start=True, stop=False)
                nc.tensor.matmul(pv[:], xt1[:, t * 128:(t + 1) * 128],
                                 wo_sb[:, 256:512], start=False, stop=True)
                nc.vector.tensor_tensor(vA[:, t * C:(t + 1) * C], pv[:],
                                        bo16_sb[:], mybir.AluOpType.add)

            # ---- v3 pieces: [105, 2 b x 2 halves x 8 a x 256 c] fp16 ----
            # partition s*35+wt = vb[h-1+s, wt+WOFF], vb = v + bo with bo
            # padding outside the image; h = hbase + 2a + b. The (b,hf,a,c)
            # free order makes each (sl,hf,b) repack DMA one contiguous
            # 4KB run per partition on both sides (consecutive vA chunks).
            # 16-row pieces (4/sample) keep the pool rotation fine-grained
            # so the second sample's repacks overlap early reassembly.
            v3s[s] = []
            vv = vA[:].rearrange("(e w) (t c) -> e w t c", e=2, c=C)
            bfc = bofill[:].rearrange("p (x b a c) -> p x b a c",
                                      x=2, b=2, a=8)
            veng = (nc.gpsimd, nc.sync, nc.scalar)
            vi = [s]

            def vdma(dst, src):
                veng[vi[0] % 3].dma_start(dst, src)
                vi[0] += 1

            for piece in range(4):
                hbase = piece * PH
                v3 = v3_pool.tile([NQ, PH * 2 * C], F16, tag="v3")
                v3s[s].append(v3)
                v3r = v3[:].rearrange("(sl w) (b hf a c) -> sl w b hf a c",
                                      sl=3, b=2, hf=2, a=8)
                # bo pad columns: hf0 wt=0 (w_v=-1), hf1 wt=34 (w_v=64)
                vdma(v3r[:, 0, :, 0, :, :], bfc[0:3, 0])
                vdma(v3r[:, 34, :, 1, :, :], bfc[0:3, 0])
                # bo pad rows (r=-1 / r=64)
                if piece == 0:   # h=0: slot0, block (b=0, hf, a=0)
                    vdma(v3r[0, :, 0, :, 0, :], bfc[0:35, 0, :, 0, :])
                if piece == 3:   # h=63: slot2, block (b=1, hf, a=7)
                    vdma(v3r[2, :, 1, :, 7, :], bfc[0:35, 0, :, 0, :])
                for sl in range(3):
                    for hf in range(2):
                        # tile w cols wt (w_v = wt + WOFF in 0..63)
                        wlo = 1 if hf == 0 else 0
                        wsrc = 0 if hf == 0 else 30
                        for b in range(2):
                            # h = hbase + 2a + b, v row r = h - 1 + sl
                            e_src = (b + sl + 1) % 2
                            alo, ahi = 0, 8
                            r0 = hbase + b + sl - 1          # r at a=0
                            if r0 < 0:
                                alo = 1                      # h=0: bo row
                            if r0 + 14 > 63:
                                ahi = 7                      # h=63: bo row
                            t0 = (r0 + 2 * alo) // 2
                            vdma(
                                v3r[sl, wlo:wlo + 34, b, hf, alo:ahi, :],
                                vv[e_src, wsrc:wsrc + 34,
                                   t0:t0 + ahi - alo, :])

        def reasm_row(s, h, r):
            if "reasm" in ablate:
                return
            krep = kreps[s]
            piece, hb = h // PH, h % PH
            mkt = mkt_pool.tile([112, 256], F16, tag="mkt")
            if "scatter" not in ablate:
                nc.gpsimd.local_scatter(
                    mkt[:],
                    krep[:, h * NIDX:(h + 1) * NIDX],
                    idx_sb[:],
                    channels=112, num_elems=256, num_idxs=NIDX)
            else:
                nc.gpsimd.memset(mkt[0:1, 0:2], 0.0)
            po = ps_o.tile([128, 512], F32, tag="pso")
            v3 = v3s[s][piece]
            a, b = hb // 2, hb % 2
            for hf in range(2):
                blk = (b * 2 + hf) * 8 + a
                nc.tensor.matmul(
                    po[:, hf * C:(hf + 1) * C],
                    mkt[0:NQ, hf * 128:(hf + 1) * 128],
                    v3[0:NQ, blk * C:(blk + 1) * C],
                    start=True, stop=True)
            ob = out_pool.tile([128, 512], F16, tag="ob")
            if r % 2 == 0:
                nc.vector.tensor_copy(ob[:], po[:])
            else:
                nc.scalar.activation(ob[:], po[:],
                                     mybir.ActivationFunctionType.Copy)
            # out row (2h+uh)*128 + 64*hf + p', p' = 2d+uw = partition%64
            engs = ((nc.sync, nc.scalar) if r % 2 == 0
                    else (nc.sync, nc.gpsimd))
            for uh in range(2):
                dst = out[s, (2 * h + uh) * 128:(2 * h + uh) * 128 + 128,
                          :].rearrange("(hf p) c -> p hf c", hf=2)
                engs[uh].dma_start(
                    dst,
                    ob[64 * uh:64 * uh + 64, :]
                    .rearrange("p (hf c) -> p hf c", hf=2))

        for _ in range(nrep):
            for s in range(BPC):
                front(s)
            for h in range(H):
                for s in range(BPC):
                    reasm_row(s, h, 2 * h + s)


# ---------------------------------------------------------------------------
# host entry
# ---------------------------------------------------------------------------

def _pack_weights(down_w, down_b, enc_w, enc_b, out_w, out_b):
    wd = np.zeros((128, 128), np.float32)
    wdT = down_w[:, :, 0, 0].T.astype(np.float32)       # [256 c, 64]
    wd[:, 0:64] = wdT[0:128]
    wd[:, 64:128] = wdT[128:256]
    weA = np.zeros((128, 108), np.float32)
    weB = np.zeros((C4, 108), np.float32)
    for dj in range(3):
        weA[0:64, dj * 36:(dj + 1) * 36] = enc_w[:, :, 0, dj].T
        weA[64:128, dj * 36:(dj + 1) * 36] = enc_w[:, :, 1, dj].T
        weB[:, dj * 36:(dj + 1) * 36] = enc_w[:, :, 2, dj].T
    woT = out_w[:, :, 0, 0].T.astype(np.float32)        # [256 c, 256 cout]
    wo = np.zeros((128, 512), np.float32)
    wo[:, 0:256] = woT[0:128]
    wo[:, 256:512] = woT[128:256]
    bo16 = np.broadcast_to(out_b.reshape(1, C), (128, C)).astype(np.float16)
    bofill = np.tile(out_b.astype(np.float16), (35, 32)).reshape(35, 16 * 2 * C)
    return {
        "wd": wd.astype(np.float16),
        "bd": down_b.reshape(C4, 1).astype(np.float32),
        "weA": weA.astype(np.float16), "weB": weB.astype(np.float16),
        "be": enc_b.reshape(NCH, 1).astype(np.float32),
        "wo": wo.astype(np.float16),
        "bo16": np.ascontiguousarray(bo16),
        "bofill": np.ascontiguousarray(bofill),
    }


def kernel(x, down_w, down_b, enc_w, enc_b, out_w, out_b):
    global last_result
    if "nc" not in _cache:
        _cache["nc"] = _build_program()
    nc = _cache["nc"]

    x = np.ascontiguousarray(np.asarray(x, np.float32).astype(np.float16))
    shared = _pack_weights(np.asarray(down_w), np.asarray(down_b),
                           np.asarray(enc_w), np.asarray(enc_b),
                           np.asarray(out_w), np.asarray(out_b))
    in_maps = []
    for i in range(NCORES):
        m = dict(shared)
        m["x2"] = np.ascontiguousarray(x[BPC * i:BPC * (i + 1)])
        in_maps.append(m)

    res = run_bass_kernel_spmd(nc, in_maps, core_ids=list(range(NCORES)),
                               trace=bool(os.environ.get("KTRACE")))
    last_result = res
    return np.concatenate([r["out"] for r in res.results],
                          axis=0).astype(np.float32)



# revision 33
# speedup vs baseline: 1.0242x; 1.0242x over previous
"""CARAFE forward as a Bass/Tile kernel for 8 Trainium2 NeuronCores.

Problem (per sample, B=16 total, data-parallel 2 samples/core):
  x [4096, 256] -> down 1x1 conv (256->64) -> enc 3x3 conv (64->36)
  -> pixel_shuffle + softmax over 9 patch weights per upsampled pixel
  -> content-adaptive reassembly of out-conv features -> out [16384, 256]

Key algebraic fusion: the trailing 1x1 conv (out_w) commutes with the
reassembly, so we conv first on the 4096-pixel grid (v = x @ wo), then
reassemble v directly into the final output: 4x fewer conv FLOPs.

Reassembly mapping (one 105-partition matmul per output row per w-half):
the contraction dim packs 3 v-rows x 35 w-columns of vb = v + bo, where
columns/rows outside the image hold bo itself -- softmax weights sum to
1, so reassembling vb with bo padding yields reassemble(v) + bo exactly,
borders included, and the out-conv bias needs no separate add. Two
overlapping w-half tiles cover the 3x3 w-taps without cross-tile halo.
The banded stationary M [112, 256] is built per row by one gpsimd
local_scatter from a replicated-kt tile (ktrep) that 36 small
partition-shifted SBUF DMAs assemble from the softmaxed kt; v3 tiles
are assembled from vA by ~26 batched partition-remapping SBUF DMAs.

Emission order runs both samples' front-ends before either reassembly so
the PE never stalls on the softmax/scatter dependency chain.
"""
import os
import sys
import numpy as np

sys.path.insert(0, "/opt/trn_rl_repo")

import concourse.bass as bass
import concourse.mybir as mybir
import concourse.tile as tile
from concourse.bass_utils import run_bass_kernel_spmd

F32 = mybir.dt.float32
F16 = mybir.dt.float16
I16 = mybir.dt.int16

U, K, C, H, W = 2, 3, 256, 64, 64
HW = H * W                      # 4096
C4 = C // 4                     # 64
NK, NU, NCH = 9, 4, 36
NCORES = 8
BPC = 2                         # samples per core

PH = 32                         # v3 piece rows (2 pieces per sample)
NQ = 105                        # live v3 partitions: 3 slots x 35 w (incl bo pad)
NIDX = 24                       # scatter idxs per partition: 2 halves x 3 j x 4 u

_cache = {}
last_result = None


# ---------------------------------------------------------------------------
# host-side constant tables
# ---------------------------------------------------------------------------

def _build_idx_tables():
    """[112, 24] int16 scatter indices (single variant).

    Partition q = s*35 + wt (s = v-row slot = tap i, wt = tile w col;
    wt column holds v value at w_v = wt + WOFF, WOFF = -1 / +30, with
    bo padding outside the image so softmax weight-sum-1 supplies the
    out-conv bias). Entry jj = half*12 + j*4 + u scatters ktrep value
    kt[h, w, 3s+j, u] (w = wt-j / wt+31-j) to M col
    half*128 + 64*(u//2) + 2*d + u%2, d = w - 32*half; -1 when the out
    pixel w falls outside the half.
    """
    idx = np.full((112, NIDX), -1, np.int16)
    for q in range(NQ):
        s, wt = q // 35, q % 35
        for half in (0, 1):
            for j in range(3):
                w = wt - j if half == 0 else wt + 31 - j
                if not 32 * half <= w < 32 * half + 32:
                    continue
                d = w - 32 * half
                for u in range(NU):
                    idx[q, half * 12 + j * 4 + u] = \
                        half * 128 + 64 * (u // 2) + 2 * d + (u % 2)
    return idx


# ---------------------------------------------------------------------------
# device program
# ---------------------------------------------------------------------------

def _build_program():
    nc = bass.Bass()

    x2 = nc.declare_dram_parameter("x2", [BPC, HW, C], F16, isOutput=False)
    wd = nc.declare_dram_parameter("wd", [128, 128], F16, isOutput=False)
    bd = nc.declare_dram_parameter("bd", [C4, 1], F32, isOutput=False)
    weA = nc.declare_dram_parameter("weA", [128, 108], F16, isOutput=False)
    weB = nc.declare_dram_parameter("weB", [C4, 108], F16, isOutput=False)
    be = nc.declare_dram_parameter("be", [NCH, 1], F32, isOutput=False)
    wo = nc.declare_dram_parameter("wo", [128, 512], F16, isOutput=False)
    bo16 = nc.declare_dram_parameter("bo16", [128, C], F16, isOutput=False)
    bofill = nc.declare_dram_parameter("bofill", [35, 16 * 2 * C], F16,
                                       isOutput=False)
    out = nc.declare_dram_parameter("out", [BPC, 4 * HW, C], F16, True)

    idx_dram = nc.inline_tensor(_build_idx_tables(), name="idxtab")
    ident_dram = nc.inline_tensor(np.eye(128, dtype=np.float32), name="ident")

    with tile.TileContext(nc) as tc:
        _emit(tc, nc, x2, wd, bd, weA, weB, be, wo, bo16, bofill, out,
              idx_dram, ident_dram)
    # raw-Bass path skips Bacc's extended-inst codegen; without this the
    # NEFF compiler sees empty .instr bytes -> "ISA wrong length"
    from concourse.library_overlay import lower_extended_insts
    lower_extended_insts(nc)
    _split_excess_waits(nc)
    return nc


def _split_excess_waits(nc, cap=1):
    """Each TPB instruction has a single EVENTS wait slot; walrus rejects
    multi-wait instructions ("Too many sync wait commands"). Move excess
    waits onto same-engine NoOps immediately before the instruction —
    semantically identical since the engine blocks at the same PC."""
    nid = [0]
    for f in nc.m.functions:
        for b in f.blocks:
            insts = b.instructions
            i = 0
            while i < len(insts):
                ins = insts[i]
                si = getattr(ins, 'sync_info', None)
                if si is not None and si.on_wait and len(si.on_wait) > cap:
                    waits = list(si.on_wait)
                    for w in waits[:-cap]:
                        nop = mybir.InstNoOp(name=f"nopw-{nid[0]}", ins=[],
                                             outs=[])
                        nid[0] += 1
                        nop.engine = ins.engine
                        nop.sync_info = mybir.SyncInfo(on_wait=[w],
                                                       on_update=[])
                        insts.insert(i, nop)
                        i += 1
                    ins.sync_info = mybir.SyncInfo(
                        on_wait=waits[-cap:],
                        on_update=list(si.on_update or []))
                i += 1


def _emit(tc, nc, x2, wd, bd, weA, weB, be, wo, bo16, bofill, out,
          idx_dram, ident_dram):
    from contextlib import ExitStack
    ablate = set(os.environ.get("KABLATE", "").split(","))
    ctx = ExitStack()
    with ctx:
        consts = ctx.enter_context(tc.tile_pool(name="consts", bufs=1))
        xt_pool = ctx.enter_context(tc.tile_pool(name="xt", bufs=1))
        o1_pool = ctx.enter_context(tc.tile_pool(name="o1", bufs=1))
        enc_pool = ctx.enter_context(tc.tile_pool(name="enc", bufs=1))
        kt_pool = ctx.enter_context(tc.tile_pool(name="kt", bufs=1))
        kr_pool = ctx.enter_context(tc.tile_pool(name="kr", bufs=2))
        v_pool = ctx.enter_context(tc.tile_pool(name="v", bufs=2))
        v3_pool = ctx.enter_context(tc.tile_pool(name="v3", bufs=3))
        mkt_pool = ctx.enter_context(tc.tile_pool(name="mkt", bufs=6))
        out_pool = ctx.enter_context(tc.tile_pool(name="ob", bufs=6))
        ps_dek = ctx.enter_context(tc.tile_pool(name="psdek", bufs=2, space="PSUM"))
        ps_v = ctx.enter_context(tc.tile_pool(name="psv", bufs=2, space="PSUM"))
        ps_o = ctx.enter_context(tc.tile_pool(name="pso", bufs=4, space="PSUM"))

        # ---- constants to SBUF ----
        wd_sb = consts.tile([128, 128], F16)
        nc.sync.dma_start(wd_sb[:], wd[:])
        bd_sb = consts.tile([C4, 1], F32)
        nc.sync.dma_start(bd_sb[:], bd[:])
        weA_sb = consts.tile([128, 108], F16)
        nc.sync.dma_start(weA_sb[:], weA[:])
        weB_sb = consts.tile([C4, 108], F16)
        nc.sync.dma_start(weB_sb[:], weB[:])
        be_sb = consts.tile([NCH, 1], F32)
        nc.sync.dma_start(be_sb[:], be[:])
        wo_sb = consts.tile([128, 512], F16)
        nc.sync.dma_start(wo_sb[:], wo[:])
        idx_sb = consts.tile([112, NIDX], I16)
        nc.sync.dma_start(idx_sb[:], idx_dram[:])
        bo16_sb = consts.tile([128, C], F16)
        nc.sync.dma_start(bo16_sb[:], bo16[:])
        id_sb = consts.tile([128, 128], F32)
        nc.sync.dma_start(id_sb[:], ident_dram[:])

        from concourse import library_config
        nc.gpsimd.load_library(library_config.local_scatter)

        nrep = int(os.environ.get("KREPEAT", "1"))
        xts, vAs, v3s, kreps = {}, {}, {}, {}

        def front(s):
            # ---- xT [256 c, 4096 pos] via DMA XBAR transpose from DRAM ----
            xt0 = xt_pool.tile([128, HW], F16, tag="xt0")
            xt1 = xt_pool.tile([128, HW], F16, tag="xt1")
            xts[s] = (xt0, xt1)
            # NOTE: both on ONE engine: concurrent XBAR transposes from two
            # engines garble each other (shared xbar state); same-engine
            # back-to-back is safe. Keep un-chunked: a row-sliced src
            # mis-lowers (emitted src pattern spans the full tensor).
            nc.sync.dma_start_transpose(xt0[:], x2[s, :, 0:128])
            nc.sync.dma_start_transpose(xt1[:], x2[s, :, 128:256])

            # ---- out1d: padded + row-shift-duplicated down-conv output ----
            o1 = o1_pool.tile([128, 66 * 66], F16, tag="o1")
            o1v = o1[:].rearrange("p (r c) -> p r c", r=66)
            nc.vector.memset(o1v[:, 0:1, :], 0.0)
            nc.vector.memset(o1v[:, 65:66, :], 0.0)
            nc.vector.memset(o1v[:, :, 0:1], 0.0)
            nc.vector.memset(o1v[:, :, 65:66], 0.0)

            for n in range(8):          # 8 h-rows per 512-chunk
                pd = ps_dek.tile([C4, 512], F32, tag="psdek")
                nc.tensor.matmul(pd[:], wd_sb[:, 0:64],
                                 xt0[:, n * 512:(n + 1) * 512],
                                 start=True, stop=False)
                nc.tensor.matmul(pd[:], wd_sb[:, 64:128],
                                 xt1[:, n * 512:(n + 1) * 512],
                                 start=False, stop=True)
                pdv = pd[:].rearrange("p (r c) -> p r c", r=8)
                nc.vector.tensor_scalar_add(
                    o1v[0:64, 1 + n * 8:9 + n * 8, 1:65], pdv, bd_sb[:])
                nc.vector.tensor_scalar_add(
                    o1v[64:128, n * 8:8 + n * 8, 1:65], pdv, bd_sb[:])

            # ---- enc conv -> enc_out [36, 4096] ----
            enc_sb = enc_pool.tile([NCH, HW], F32, tag="enc")
            for n in range(8):
                pe = ps_dek.tile([NCH, 512], F32, tag="psdek")
                first = True
                for dj in range(3):
                    nc.tensor.matmul(
                        pe[:], weA_sb[:, dj * 36:(dj + 1) * 36],
                        o1v[:, n * 8:n * 8 + 8, dj:dj + 64],
                        start=first, stop=False)
                    first = False
                    nc.tensor.matmul(
                        pe[:], weB_sb[:, dj * 36:(dj + 1) * 36],
                        o1v[0:64, n * 8 + 2:n * 8 + 10, dj:dj + 64],
                        start=False, stop=(dj == 2))
                nc.vector.tensor_scalar_add(
                    enc_sb[:, n * 512:(n + 1) * 512], pe[:], be_sb[:])

            # ---- kt transpose: [4096 pos, 36] chunks + softmax ----
            kts = kt_pool.tile([128, 32 * NCH], F32, tag="kts")
            for c in range(32):
                pt = ps_dek.tile([128, NCH], F32, tag="psdek")
                nc.tensor.matmul(pt[:], enc_sb[:, c * 128:(c + 1) * 128],
                                 id_sb[0:NCH, 0:NCH], is_transpose=True)
                nc.vector.tensor_copy(kts[:, c * NCH:(c + 1) * NCH], pt[:])

            ea = kt_pool.tile([128, 32 * NCH], F32, tag="ea")
            nc.scalar.activation(ea[:], kts[:],
                                 mybir.ActivationFunctionType.Exp)
            sums = kt_pool.tile([128, 128], F32, tag="sums")
            nc.vector.reduce_sum(sums[:].rearrange("p (c u) -> p c u", u=NU),
                                 ea[:].rearrange("p (c k u) -> p c u k",
                                                 k=NK, u=NU),
                                 axis=mybir.AxisListType.X)
            rec = kt_pool.tile([128, 128], F32, tag="rec")
            nc.vector.reciprocal(rec[:], sums[:])
            # kt16 [128 pos%128, 32 chunks x (k,u)] fp16 softmaxed weights
            kt16 = kt_pool.tile([128, 32 * NCH], F16, tag="kt16")
            recb = rec[:].rearrange("p (c u) -> p c u", u=NU) \
                         .unsqueeze(2).broadcast_to([128, 32, NK, NU])
            nc.vector.tensor_tensor(
                kt16[:].rearrange("p (c k u) -> p c k u", k=NK, u=NU),
                ea[:].rearrange("p (c k u) -> p c k u", k=NK, u=NU),
                recb, mybir.AluOpType.mult)

            # ---- ktrep [112, 64 h x 24]: per-partition scatter source ----
            # ktrep[s*35+wt, h*24 + hf*12 + j*4 + u] = kt[h, w, 3s+j, u],
            # w = wt - j (hf=0) / wt + 31 - j (hf=1)
            krep = kr_pool.tile([112, 64 * NIDX], F16, tag="krep")
            kreps[s] = krep
            kv = kt16[:].rearrange("(e w) (hp k u) -> e w hp k u",
                                   e=2, k=NK, u=NU)
            dv = krep[:].rearrange("q (hp e hf j u) -> q hp e hf j u",
                                   hp=32, e=2, hf=2, j=3)
            for sl in range(3):
                for hf in range(2):
                    for j in range(3):
                        off = -j if hf == 0 else 31 - j
                        wlo = j if hf == 0 else j + 1
                        kk = 3 * sl + j
                        for e in range(2):
                            # keep sync free: it owns the (serialized) XBAR
                            # transposes that gate the next sample's front
                            eng = (nc.gpsimd,
                                   nc.scalar)[(2 * hf + j + e) % 2]
                            eng.dma_start(
                                dv[sl * 35 + wlo:sl * 35 + wlo + 32,
                                   :, e, hf, j, :],
                                kv[e, wlo + off:wlo + off + 32, :, kk, :])

            # ---- v = x @ wo (bias via reassembly bias row) ----
            vA = v_pool.tile([128, 32 * C], F16, tag="vA")
            vAs[s] = vA
            for t in range(32):
                pv = ps_v.tile([128, C], F32, tag="psv")
                nc.tensor.matmul(pv[:], xt0[:, t * 128:(t + 1) * 128],
                                 wo_sb[:, 0:256], start=True, stop=False)
                nc.tensor.matmul(pv[:], xt1[:, t * 128:(t + 1) * 128],
                                 wo_sb[:, 256:512], start=False, stop=True)
                nc.vector.tensor_tensor(vA[:, t * C:(t + 1) * C], pv[:],
                                        bo16_sb[:], mybir.AluOpType.add)

            # ---- v3 pieces: [105, 2 b x 2 halves x 16 a x 256 c] fp16 ----
            # partition s*35+wt = vb[h-1+s, wt+WOFF], vb = v + bo with bo
            # padding outside the image; h = hbase + 2a + b. The (b,hf,a,c)
            # free order makes each (sl,hf,b) repack DMA one contiguous
            # 8KB run per partition on both sides (consecutive vA chunks).
            v3s[s] = []
            vv = vA[:].rearrange("(e w) (t c) -> e w t c", e=2, c=C)
            bfc = bofill[:].rearrange("p (b a c) -> p b a c", b=2, a=16)
            for piece in range(2):
                hbase = piece * PH
                v3 = v3_pool.tile([NQ, PH * 2 * C], F16, tag="v3")
                v3s[s].append(v3)
                v3r = v3[:].rearrange("(sl w) (b hf a c) -> sl w b hf a c",
                                      sl=3, b=2, hf=2, a=16)
                # bo pad columns: hf0 wt=0 (w_v=-1), hf1 wt=34 (w_v=64)
                v3eng = ((nc.gpsimd, nc.gpsimd) if s == 0
                         else (nc.sync, nc.scalar))
                v3eng[0].dma_start(v3r[:, 0, :, 0, :, :], bfc[0:3])
                v3eng[1].dma_start(v3r[:, 34, :, 1, :, :], bfc[0:3])
                # bo pad rows (r=-1 / r=64)
                if piece == 0:   # h=0: slot0, block (b=0, hf, a=0)
                    v3eng[0].dma_start(v3r[0, :, 0, :, 0, :],
                                       bfc[0:35, :, 0, :])
                else:            # h=63: slot2, block (b=1, hf, a=15)
                    v3eng[1].dma_start(v3r[2, :, 1, :, 15, :],
                                       bfc[0:35, :, 0, :])
                for sl in range(3):
                    for hf in range(2):
                        # tile w cols wt (w_v = wt + WOFF in 0..63)
                        wlo = 1 if hf == 0 else 0
                        wsrc = 0 if hf == 0 else 30
                        for b in range(2):
                            # h = hbase + 2a + b, v row r = h - 1 + sl
                            e_src = (b + sl + 1) % 2
                            alo, ahi = 0, 16
                            r0 = hbase + b + sl - 1          # r at a=0
                            if r0 < 0:
                                alo = 1                      # h=0: bo row
                            if r0 + 30 > 63:
                                ahi = 15                     # h=63: bo row
                            t0 = (r0 + 2 * alo) // 2
                            v3eng[(sl + b) % 2].dma_start(
                                v3r[sl, wlo:wlo + 34, b, hf, alo:ahi, :],
                                vv[e_src, wsrc:wsrc + 34,
                                   t0:t0 + ahi - alo, :])

        def reasm_row(s, h, r):
            if "reasm" in ablate:
                return
            krep = kreps[s]
            piece, hb = h // PH, h % PH
            mkt = mkt_pool.tile([112, 256], F16, tag="mkt")
            if "scatter" not in ablate:
                nc.gpsimd.local_scatter(
                    mkt[:],
                    krep[:, h * NIDX:(h + 1) * NIDX],
                    idx_sb[:],
                    channels=112, num_elems=256, num_idxs=NIDX)
            else:
                nc.gpsimd.memset(mkt[0:1, 0:2], 0.0)
            po = ps_o.tile([128, 512], F32, tag="pso")
            v3 = v3s[s][piece]
            a, b = hb // 2, hb % 2
            for hf in range(2):
                blk = (b * 2 + hf) * 16 + a
                nc.tensor.matmul(
                    po[:, hf * C:(hf + 1) * C],
                    mkt[0:NQ, hf * 128:(hf + 1) * 128],
                    v3[0:NQ, blk * C:(blk + 1) * C],
                    start=True, stop=True)
            ob = out_pool.tile([128, 512], F16, tag="ob")
            if r % 2 == 0:
                nc.vector.tensor_copy(ob[:], po[:])
            else:
                nc.scalar.activation(ob[:], po[:],
                                     mybir.ActivationFunctionType.Copy)
            # out row (2h+uh)*128 + 64*hf + p', p' = 2d+uw = partition%64
            engs = ((nc.sync, nc.scalar) if r % 2 == 0
                    else (nc.sync, nc.gpsimd))
            for uh in range(2):
                dst = out[s, (2 * h + uh) * 128:(2 * h + uh) * 128 + 128,
                          :].rearrange("(hf p) c -> p hf c", hf=2)
                engs[uh].dma_start(
                    dst,
                    ob[64 * uh:64 * uh + 64, :]
                    .rearrange("p (hf c) -> p hf c", hf=2))

        # s0 leads s1 by LAG rows so the v3 pool rotation (s1's piece
        # reusing s0's slot) has drained before s1's rows need it.
        LAG = 16
        for _ in range(nrep):
            for s in range(BPC):
                front(s)
            rows = [(0, h) for h in range(LAG)]
            for i in range(H - LAG):
                rows.append((0, LAG + i))
                rows.append((1, i))
            rows += [(1, h) for h in range(H - LAG, H)]
            for r, (s, h) in enumerate(rows):
                reasm_row(s, h, r)


# ---------------------------------------------------------------------------
# host entry
# ---------------------------------------------------------------------------

def _pack_weights(down_w, down_b, enc_w, enc_b, out_w, out_b):
    wd = np.zeros((128, 128), np.float32)
    wdT = down_w[:, :, 0, 0].T.astype(np.float32)       # [256 c, 64]
    wd[:, 0:64] = wdT[0:128]
    wd[:, 64:128] = wdT[128:256]
    weA = np.zeros((128, 108), np.float32)
    weB = np.zeros((C4, 108), np.float32)
    for dj in range(3):
        weA[0:64, dj * 36:(dj + 1) * 36] = enc_w[:, :, 0, dj].T
        weA[64:128, dj * 36:(dj + 1) * 36] = enc_w[:, :, 1, dj].T
        weB[:, dj * 36:(dj + 1) * 36] = enc_w[:, :, 2, dj].T
    woT = out_w[:, :, 0, 0].T.astype(np.float32)        # [256 c, 256 cout]
    wo = np.zeros((128, 512), np.float32)
    wo[:, 0:256] = woT[0:128]
    wo[:, 256:512] = woT[128:256]
    bo16 = np.broadcast_to(out_b.reshape(1, C), (128, C)).astype(np.float16)
    bofill = np.tile(out_b.astype(np.float16), (35, 32)).reshape(35, 16 * 2 * C)
    return {
        "wd": wd.astype(np.float16),
        "bd": down_b.reshape(C4, 1).astype(np.float32),
        "weA": weA.astype(np.float16), "weB": weB.astype(np.float16),
        "be": enc_b.reshape(NCH, 1).astype(np.float32),
        "wo": wo.astype(np.float16),
        "bo16": np.ascontiguousarray(bo16),
        "bofill": np.ascontiguousarray(bofill),
    }


def kernel(x, down_w, down_b, enc_w, enc_b, out_w, out_b):
    global last_result
    if "nc" not in _cache:
        _cache["nc"] = _build_program()
    nc = _cache["nc"]

    x = np.ascontiguousarray(np.asarray(x, np.float32).astype(np.float16))
    shared = _pack_weights(np.asarray(down_w), np.asarray(down_b),
                           np.asarray(enc_w), np.asarray(enc_b),
                           np.asarray(out_w), np.asarray(out_b))
    in_maps = []
    for i in range(NCORES):
        m = dict(shared)
        m["x2"] = np.ascontiguousarray(x[BPC * i:BPC * (i + 1)])
        in_maps.append(m)

    res = run_bass_kernel_spmd(nc, in_maps, core_ids=list(range(NCORES)),
                               trace=bool(os.environ.get("KTRACE")))
    last_result = res
    return np.concatenate([r["out"] for r in res.results],
                          axis=0).astype(np.float32)



# revision 56
# speedup vs baseline: 1.0925x; 1.0668x over previous
"""CARAFE forward as a Bass/Tile kernel for 8 Trainium2 NeuronCores.

Problem (per sample, B=16 total, data-parallel 2 samples/core):
  x [4096, 256] -> down 1x1 conv (256->64) -> enc 3x3 conv (64->36)
  -> pixel_shuffle + softmax over 9 patch weights per upsampled pixel
  -> content-adaptive reassembly of out-conv features -> out [16384, 256]

Key algebraic fusion: the trailing 1x1 conv (out_w) commutes with the
reassembly, so we conv first on the 4096-pixel grid (v = x @ wo), then
reassemble v directly into the final output: 4x fewer conv FLOPs.

Reassembly mapping (one 105-partition matmul per output row per w-half):
the contraction dim packs 3 v-rows x 35 w-columns of vb = v + bo, where
columns/rows outside the image hold bo itself -- softmax weights sum to
1, so reassembling vb with bo padding yields reassemble(v) + bo exactly,
borders included, and the out-conv bias needs no separate add. Two
overlapping w-half tiles cover the 3x3 w-taps without cross-tile halo.
The banded stationary M [112, 256] is built per row by one gpsimd
local_scatter from a replicated-kt tile (ktrep) that 36 small
partition-shifted SBUF DMAs assemble from the softmaxed kt; v3 tiles
are assembled from vA by ~26 batched partition-remapping SBUF DMAs.

Emission order runs both samples' front-ends before either reassembly so
the PE never stalls on the softmax/scatter dependency chain.
"""
import os
import sys
import numpy as np

sys.path.insert(0, "/opt/trn_rl_repo")

import concourse.bass as bass
import concourse.mybir as mybir
import concourse.tile as tile
from concourse.bass_utils import run_bass_kernel_spmd

F32 = mybir.dt.float32
F16 = mybir.dt.float16
I16 = mybir.dt.int16

U, K, C, H, W = 2, 3, 256, 64, 64
HW = H * W                      # 4096
C4 = C // 4                     # 64
NK, NU, NCH = 9, 4, 36
NCORES = 8
BPC = 2                         # samples per core

PH = 32                         # v3 piece rows (2 pieces per sample)
NQ = 105                        # live v3 partitions: 3 slots x 35 w (incl bo pad)
NIDX = 24                       # scatter idxs per partition: 2 halves x 3 j x 4 u

_cache = {}
last_result = None


# ---------------------------------------------------------------------------
# host-side constant tables
# ---------------------------------------------------------------------------

def _build_idx_tables():
    """[112, 24] int16 scatter indices (single variant).

    Partition q = s*35 + wt (s = v-row slot = tap i, wt = tile w col;
    wt column holds v value at w_v = wt + WOFF, WOFF = -1 / +30, with
    bo padding outside the image so softmax weight-sum-1 supplies the
    out-conv bias). Entry jj = half*12 + j*4 + u scatters ktrep value
    kt[h, w, 3s+j, u] (w = wt-j / wt+31-j) to M col
    half*128 + 64*(u//2) + 2*d + u%2, d = w - 32*half; -1 when the out
    pixel w falls outside the half.
    """
    idx = np.full((112, NIDX), -1, np.int16)
    for q in range(NQ):
        s, wt = q // 35, q % 35
        for half in (0, 1):
            for j in range(3):
                w = wt - j if half == 0 else wt + 31 - j
                if not 32 * half <= w < 32 * half + 32:
                    continue
                d = w - 32 * half
                for u in range(NU):
                    idx[q, half * 12 + j * 4 + u] = \
                        half * 128 + 64 * (u // 2) + 2 * d + (u % 2)
    # pair-batched scatter: one gpsimd scatter fills M for rows (h, h+1)
    # into a [112, 512] tile; odd-row entries target the +256 half
    return np.concatenate([idx, np.where(idx < 0, idx, idx + 256)], axis=1)


# ---------------------------------------------------------------------------
# device program
# ---------------------------------------------------------------------------

def _build_program():
    nc = bass.Bass()

    x2 = nc.declare_dram_parameter("x2", [BPC, HW, C], F16, isOutput=False)
    wd = nc.declare_dram_parameter("wd", [128, 128], F16, isOutput=False)
    bd = nc.declare_dram_parameter("bd", [C4, 1], F32, isOutput=False)
    weA = nc.declare_dram_parameter("weA", [128, 108], F16, isOutput=False)
    weB = nc.declare_dram_parameter("weB", [C4, 108], F16, isOutput=False)
    be = nc.declare_dram_parameter("be", [NCH, 1], F32, isOutput=False)
    wo = nc.declare_dram_parameter("wo", [128, 512], F16, isOutput=False)
    bo16 = nc.declare_dram_parameter("bo16", [128, C], F16, isOutput=False)
    bofill = nc.declare_dram_parameter("bofill", [35, 16 * 2 * C], F16,
                                       isOutput=False)
    out = nc.declare_dram_parameter("out", [BPC, 4 * HW, C], F16, True)

    idx_dram = nc.inline_tensor(_build_idx_tables(), name="idxtab")
    ident_dram = nc.inline_tensor(np.eye(128, dtype=np.float32), name="ident")
    # single-diagonal const: E[w, c] = 1 iff c == w + 72; column slices of
    # it give every banded shift stationary the PE ktrep build needs
    # two 128-row diagonal consts (diag at c = p%64 + 72), one per w-half:
    # L keeps rows with p%64 < 32, H keeps p%64 >= 32. Slicing 64 rows at
    # base e*64 then restricts the contraction to one (e, hf) w-quarter.
    p = np.arange(128)
    ediag = np.zeros((2, 128, 216), np.float16)
    ediag[(p % 64) // 32, p, (p % 64) + 72] = 1.0
    ediag_dram = nc.inline_tensor(
        np.concatenate([ediag[0], ediag[1]], axis=1), name="ediag")

    with tile.TileContext(nc) as tc:
        _emit(tc, nc, x2, wd, bd, weA, weB, be, wo, bo16, bofill, out,
              idx_dram, ident_dram, ediag_dram)
    # raw-Bass path skips Bacc's extended-inst codegen; without this the
    # NEFF compiler sees empty .instr bytes -> "ISA wrong length"
    from concourse.library_overlay import lower_extended_insts
    lower_extended_insts(nc)
    _split_excess_waits(nc)
    return nc


def _split_excess_waits(nc, cap=1):
    """Each TPB instruction has a single EVENTS wait slot; walrus rejects
    multi-wait instructions ("Too many sync wait commands"). Move excess
    waits onto same-engine NoOps immediately before the instruction —
    semantically identical since the engine blocks at the same PC."""
    nid = [0]
    for f in nc.m.functions:
        for b in f.blocks:
            insts = b.instructions
            i = 0
            while i < len(insts):
                ins = insts[i]
                si = getattr(ins, 'sync_info', None)
                if si is not None and si.on_wait and len(si.on_wait) > cap:
                    waits = list(si.on_wait)
                    for w in waits[:-cap]:
                        nop = mybir.InstNoOp(name=f"nopw-{nid[0]}", ins=[],
                                             outs=[])
                        nid[0] += 1
                        nop.engine = ins.engine
                        nop.sync_info = mybir.SyncInfo(on_wait=[w],
                                                       on_update=[])
                        insts.insert(i, nop)
                        i += 1
                    ins.sync_info = mybir.SyncInfo(
                        on_wait=waits[-cap:],
                        on_update=list(si.on_update or []))
                i += 1


def _emit(tc, nc, x2, wd, bd, weA, weB, be, wo, bo16, bofill, out,
          idx_dram, ident_dram, ediag_dram):
    from contextlib import ExitStack
    ablate = set(os.environ.get("KABLATE", "").split(","))
    ctx = ExitStack()
    with ctx:
        consts = ctx.enter_context(tc.tile_pool(name="consts", bufs=1))
        xt_pool = ctx.enter_context(tc.tile_pool(name="xt", bufs=1))
        o1_pool = ctx.enter_context(tc.tile_pool(name="o1", bufs=1))
        enc_pool = ctx.enter_context(tc.tile_pool(name="enc", bufs=1))
        kt_pool = ctx.enter_context(tc.tile_pool(name="kt", bufs=1))
        kr_pool = ctx.enter_context(tc.tile_pool(name="kr", bufs=2))
        v_pool = ctx.enter_context(tc.tile_pool(name="v", bufs=2))
        v3_pool = ctx.enter_context(tc.tile_pool(name="v3", bufs=3))
        mkt_pool = ctx.enter_context(tc.tile_pool(name="mkt", bufs=4))
        out_pool = ctx.enter_context(tc.tile_pool(name="ob", bufs=6))
        ps_dek = ctx.enter_context(tc.tile_pool(name="psdek", bufs=2, space="PSUM"))
        ps_v = ctx.enter_context(tc.tile_pool(name="psv", bufs=2, space="PSUM"))
        ps_o = ctx.enter_context(tc.tile_pool(name="pso", bufs=4, space="PSUM"))

        # ---- constants to SBUF ----
        wd_sb = consts.tile([128, 128], F16)
        nc.sync.dma_start(wd_sb[:], wd[:])
        bd_sb = consts.tile([C4, 1], F32)
        nc.sync.dma_start(bd_sb[:], bd[:])
        weA_sb = consts.tile([128, 108], F16)
        nc.sync.dma_start(weA_sb[:], weA[:])
        weB_sb = consts.tile([C4, 108], F16)
        nc.sync.dma_start(weB_sb[:], weB[:])
        be_sb = consts.tile([NCH, 1], F32)
        nc.sync.dma_start(be_sb[:], be[:])
        wo_sb = consts.tile([128, 512], F16)
        nc.sync.dma_start(wo_sb[:], wo[:])
        idx_sb = consts.tile([112, 2 * NIDX], I16)
        nc.sync.dma_start(idx_sb[:], idx_dram[:])
        ediag_sb = consts.tile([128, 432], F16)
        nc.sync.dma_start(ediag_sb[:], ediag_dram[:])
        bo16_sb = consts.tile([128, C], F16)
        nc.sync.dma_start(bo16_sb[:], bo16[:])
        id_sb = consts.tile([128, 128], F32)
        nc.sync.dma_start(id_sb[:], ident_dram[:])

        from concourse import library_config
        nc.gpsimd.load_library(library_config.local_scatter)

        nrep = int(os.environ.get("KREPEAT", "1"))
        xts, vAs, v3s, kreps = {}, {}, {}, {}

        def front(s):
            # ---- xT [256 c, 4096 pos] via DMA XBAR transpose from DRAM ----
            xt0 = xt_pool.tile([128, HW], F16, tag="xt0")
            xt1 = xt_pool.tile([128, HW], F16, tag="xt1")
            xts[s] = (xt0, xt1)
            # NOTE: both on ONE engine: concurrent XBAR transposes from two
            # engines garble each other (shared xbar state); same-engine
            # back-to-back is safe. Keep un-chunked: a row-sliced src
            # mis-lowers (emitted src pattern spans the full tensor).
            nc.sync.dma_start_transpose(xt0[:], x2[s, :, 0:128])
            nc.sync.dma_start_transpose(xt1[:], x2[s, :, 128:256])

            # ---- out1d: padded + row-shift-duplicated down-conv output ----
            o1 = o1_pool.tile([128, 66 * 66], F16, tag="o1")
            o1v = o1[:].rearrange("p (r c) -> p r c", r=66)
            nc.vector.memset(o1v[:, 0:1, :], 0.0)
            nc.vector.memset(o1v[:, 65:66, :], 0.0)
            nc.vector.memset(o1v[:, :, 0:1], 0.0)
            nc.vector.memset(o1v[:, :, 65:66], 0.0)

            for n in range(8):          # 8 h-rows per 512-chunk
                pd = ps_dek.tile([C4, 512], F32, tag="psdek")
                nc.tensor.matmul(pd[:], wd_sb[:, 0:64],
                                 xt0[:, n * 512:(n + 1) * 512],
                                 start=True, stop=False)
                nc.tensor.matmul(pd[:], wd_sb[:, 64:128],
                                 xt1[:, n * 512:(n + 1) * 512],
                                 start=False, stop=True)
                pdv = pd[:].rearrange("p (r c) -> p r c", r=8)
                nc.vector.tensor_scalar_add(
                    o1v[0:64, 1 + n * 8:9 + n * 8, 1:65], pdv, bd_sb[:])
                nc.vector.tensor_scalar_add(
                    o1v[64:128, n * 8:8 + n * 8, 1:65], pdv, bd_sb[:])

            # ---- enc conv -> enc_out [36, 4096] ----
            enc_sb = enc_pool.tile([NCH, HW], F32, tag="enc")
            for n in range(8):
                pe = ps_dek.tile([NCH, 512], F32, tag="psdek")
                first = True
                for dj in range(3):
                    nc.tensor.matmul(
                        pe[:], weA_sb[:, dj * 36:(dj + 1) * 36],
                        o1v[:, n * 8:n * 8 + 8, dj:dj + 64],
                        start=first, stop=False)
                    first = False
                    nc.tensor.matmul(
                        pe[:], weB_sb[:, dj * 36:(dj + 1) * 36],
                        o1v[0:64, n * 8 + 2:n * 8 + 10, dj:dj + 64],
                        start=False, stop=(dj == 2))
                nc.vector.tensor_scalar_add(
                    enc_sb[:, n * 512:(n + 1) * 512], pe[:], be_sb[:])

            # ---- kt transpose: [4096 pos, 36] chunks + softmax ----
            kts = kt_pool.tile([128, 32 * NCH], F32, tag="kts")
            for c in range(32):
                pt = ps_dek.tile([128, NCH], F32, tag="psdek")
                nc.tensor.matmul(pt[:], enc_sb[:, c * 128:(c + 1) * 128],
                                 id_sb[0:NCH, 0:NCH], is_transpose=True)
                nc.vector.tensor_copy(kts[:, c * NCH:(c + 1) * NCH], pt[:])

            ea = kt_pool.tile([128, 32 * NCH], F32, tag="ea")
            nc.scalar.activation(ea[:], kts[:],
                                 mybir.ActivationFunctionType.Exp)
            sums = kt_pool.tile([128, 128], F32, tag="sums")
            nc.vector.reduce_sum(sums[:].rearrange("p (c u) -> p c u", u=NU),
                                 ea[:].rearrange("p (c k u) -> p c u k",
                                                 k=NK, u=NU),
                                 axis=mybir.AxisListType.X)
            rec = kt_pool.tile([128, 128], F32, tag="rec")
            nc.vector.reciprocal(rec[:], sums[:])
            # kt16 [128 pos%128, 32 chunks x (k,u)] fp16 softmaxed weights
            kt16 = kt_pool.tile([128, 32 * NCH], F16, tag="kt16")
            recb = rec[:].rearrange("p (c u) -> p c u", u=NU) \
                         .unsqueeze(2).broadcast_to([128, 32, NK, NU])
            nc.vector.tensor_tensor(
                kt16[:].rearrange("p (c k u) -> p c k u", k=NK, u=NU),
                ea[:].rearrange("p (c k u) -> p c k u", k=NK, u=NU),
                recb, mybir.AluOpType.mult)

            # ---- ktrep [112, 64 h x 24]: per-partition scatter source ----
            # ktrep[s*35+wt, h*24 + hf*12 + j*4 + u] = kt[h, w, 3s+j, u],
            # w = wt - j (hf=0) / wt + 31 - j (hf=1)
            krep = kr_pool.tile([112, 64 * NIDX], F16, tag="krep")
            kreps[s] = krep
            kv = kt16[:].rearrange("(e w) (hp k u) -> e w hp k u",
                                   e=2, k=NK, u=NU)
            dv = krep[:].rearrange("q (hp e hf j u) -> q hp e hf j u",
                                   hp=32, e=2, hf=2, j=3)
            for sl in range(3):
                for hf in range(2):
                    for j in range(3):
                        off = -j if hf == 0 else 31 - j
                        wlo = j if hf == 0 else j + 1
                        kk = 3 * sl + j
                        for e in range(2):
                            eng = (nc.gpsimd, nc.scalar,
                                   nc.sync)[(6 * sl + 2 * hf + j + e) % 3]
                            eng.dma_start(
                                dv[sl * 35 + wlo:sl * 35 + wlo + 32,
                                   :, e, hf, j, :],
                                kv[e, wlo + off:wlo + off + 32, :, kk, :])

            # ---- v = x @ wo (bias via reassembly bias row) ----
            vA = v_pool.tile([128, 32 * C], F16, tag="vA")
            vAs[s] = vA
            for t in range(32):
                pv = ps_v.tile([128, C], F32, tag="psv")
                nc.tensor.matmul(pv[:], xt0[:, t * 128:(t + 1) * 128],
                                 wo_sb[:, 0:256], start=True, stop=False)
                nc.tensor.matmul(pv[:], xt1[:, t * 128:(t + 1) * 128],
                                 wo_sb[:, 256:512], start=False, stop=True)
                nc.vector.tensor_tensor(vA[:, t * C:(t + 1) * C], pv[:],
                                        bo16_sb[:], mybir.AluOpType.add)

            # ---- v3 pieces: [105, 2 b x 2 halves x 16 a x 256 c] fp16 ----
            # partition s*35+wt = vb[h-1+s, wt+WOFF], vb = v + bo with bo
            # padding outside the image; h = hbase + 2a + b. The (b,hf,a,c)
            # free order makes each (sl,hf,b) repack DMA one contiguous
            # 8KB run per partition on both sides (consecutive vA chunks).
            v3s[s] = []
            vv = vA[:].rearrange("(e w) (t c) -> e w t c", e=2, c=C)
            bfc = bofill[:].rearrange("p (b a c) -> p b a c", b=2, a=16)
            for piece in range(2):
                hbase = piece * PH
                v3 = v3_pool.tile([NQ, PH * 2 * C], F16, tag="v3")
                v3s[s].append(v3)
                v3r = v3[:].rearrange("(sl w) (b hf a c) -> sl w b hf a c",
                                      sl=3, b=2, hf=2, a=16)
                # bo pad columns: hf0 wt=0 (w_v=-1), hf1 wt=34 (w_v=64)
                v3eng = ((nc.gpsimd, nc.gpsimd) if s == 0
                         else (nc.sync, nc.scalar))
                v3eng[0].dma_start(v3r[:, 0, :, 0, :, :], bfc[0:3])
                v3eng[1].dma_start(v3r[:, 34, :, 1, :, :], bfc[0:3])
                # bo pad rows (r=-1 / r=64)
                if piece == 0:   # h=0: slot0, block (b=0, hf, a=0)
                    v3eng[0].dma_start(v3r[0, :, 0, :, 0, :],
                                       bfc[0:35, :, 0, :])
                else:            # h=63: slot2, block (b=1, hf, a=15)
                    v3eng[1].dma_start(v3r[2, :, 1, :, 15, :],
                                       bfc[0:35, :, 0, :])
                for sl in range(3):
                    for hf in range(2):
                        # tile w cols wt (w_v = wt + WOFF in 0..63)
                        wlo = 1 if hf == 0 else 0
                        wsrc = 0 if hf == 0 else 30
                        for b in range(2):
                            # h = hbase + 2a + b, v row r = h - 1 + sl
                            e_src = (b + sl + 1) % 2
                            alo, ahi = 0, 16
                            r0 = hbase + b + sl - 1          # r at a=0
                            if r0 < 0:
                                alo = 1                      # h=0: bo row
                            if r0 + 30 > 63:
                                ahi = 15                     # h=63: bo row
                            t0 = (r0 + 2 * alo) // 2
                            v3eng[(sl + b) % 2].dma_start(
                                v3r[sl, wlo:wlo + 34, b, hf, alo:ahi, :],
                                vv[e_src, wsrc:wsrc + 34,
                                   t0:t0 + ahi - alo, :])

        mkts = {}

        def reasm_row(s, h, r):
            if "reasm" in ablate:
                return
            krep = kreps[s]
            piece, hb = h // PH, h % PH
            if True:
                mkt = mkt_pool.tile([112, 256], F16, tag="mkt")
                mkts[s] = mkt
                if "scatter" not in ablate:
                    nc.gpsimd.local_scatter(
                        mkt[:],
                        krep[:, h * NIDX:(h + 1) * NIDX],
                        idx_sb[:, 0:NIDX],
                        channels=112, num_elems=256, num_idxs=NIDX)
                else:
                    nc.gpsimd.memset(mkt[0:1, 0:2], 0.0)
            mkt = mkts[s]
            moff = 0
            po = ps_o.tile([128, 512], F32, tag="pso")
            v3 = v3s[s][piece]
            a, b = hb // 2, hb % 2
            for hf in range(2):
                blk = (b * 2 + hf) * 16 + a
                nc.tensor.matmul(
                    po[:, hf * C:(hf + 1) * C],
                    mkt[0:NQ, moff + hf * 128:moff + (hf + 1) * 128],
                    v3[0:NQ, blk * C:(blk + 1) * C],
                    start=True, stop=True)
            ob = out_pool.tile([128, 512], F16, tag="ob")
            if r % 2 == 0:
                nc.vector.tensor_copy(ob[:], po[:])
            else:
                nc.scalar.activation(ob[:], po[:],
                                     mybir.ActivationFunctionType.Copy)
            # out row (2h+uh)*128 + 64*hf + p', p' = 2d+uw = partition%64
            engs = ((nc.sync, nc.scalar) if r % 2 == 0
                    else (nc.sync, nc.gpsimd))
            for uh in range(2):
                dst = out[s, (2 * h + uh) * 128:(2 * h + uh) * 128 + 128,
                          :].rearrange("(hf p) c -> p hf c", hf=2)
                engs[uh].dma_start(
                    dst,
                    ob[64 * uh:64 * uh + 64, :]
                    .rearrange("p (hf c) -> p hf c", hf=2))

        for _ in range(nrep):
            for s in range(BPC):
                front(s)
            for h in range(H):
                for s in range(BPC):
                    reasm_row(s, h, 2 * h + s)


# ---------------------------------------------------------------------------
# host entry
# ---------------------------------------------------------------------------

def _pack_weights(down_w, down_b, enc_w, enc_b, out_w, out_b):
    wd = np.zeros((128, 128), np.float32)
    wdT = down_w[:, :, 0, 0].T.astype(np.float32)       # [256 c, 64]
    wd[:, 0:64] = wdT[0:128]
    wd[:, 64:128] = wdT[128:256]
    weA = np.zeros((128, 108), np.float32)
    weB = np.zeros((C4, 108), np.float32)
    for dj in range(3):
        weA[0:64, dj * 36:(dj + 1) * 36] = enc_w[:, :, 0, dj].T
        weA[64:128, dj * 36:(dj + 1) * 36] = enc_w[:, :, 1, dj].T
        weB[:, dj * 36:(dj + 1) * 36] = enc_w[:, :, 2, dj].T
    woT = out_w[:, :, 0, 0].T.astype(np.float32)        # [256 c, 256 cout]
    wo = np.zeros((128, 512), np.float32)
    wo[:, 0:256] = woT[0:128]
    wo[:, 256:512] = woT[128:256]
    bo16 = np.broadcast_to(out_b.reshape(1, C), (128, C)).astype(np.float16)
    bofill = np.tile(out_b.astype(np.float16), (35, 32)).reshape(35, 16 * 2 * C)
    return {
        "wd": wd.astype(np.float16),
        "bd": down_b.reshape(C4, 1).astype(np.float32),
        "weA": weA.astype(np.float16), "weB": weB.astype(np.float16),
        "be": enc_b.reshape(NCH, 1).astype(np.float32),
        "wo": wo.astype(np.float16),
        "bo16": np.ascontiguousarray(bo16),
        "bofill": np.ascontiguousarray(bofill),
    }


def kernel(x, down_w, down_b, enc_w, enc_b, out_w, out_b):
    global last_result
    if "nc" not in _cache:
        _cache["nc"] = _build_program()
    nc = _cache["nc"]

    x = np.ascontiguousarray(np.asarray(x, np.float32).astype(np.float16))
    shared = _pack_weights(np.asarray(down_w), np.asarray(down_b),
                           np.asarray(enc_w), np.asarray(enc_b),
                           np.asarray(out_w), np.asarray(out_b))
    in_maps = []
    for i in range(NCORES):
        m = dict(shared)
        m["x2"] = np.ascontiguousarray(x[BPC * i:BPC * (i + 1)])
        in_maps.append(m)

    res = run_bass_kernel_spmd(nc, in_maps, core_ids=list(range(NCORES)),
                               trace=bool(os.environ.get("KTRACE")))
    last_result = res
    return np.concatenate([r["out"] for r in res.results],
                          axis=0).astype(np.float32)



# revision 57
# speedup vs baseline: 1.1012x; 1.0079x over previous
"""CARAFE forward as a Bass/Tile kernel for 8 Trainium2 NeuronCores.

Problem (per sample, B=16 total, data-parallel 2 samples/core):
  x [4096, 256] -> down 1x1 conv (256->64) -> enc 3x3 conv (64->36)
  -> pixel_shuffle + softmax over 9 patch weights per upsampled pixel
  -> content-adaptive reassembly of out-conv features -> out [16384, 256]

Key algebraic fusion: the trailing 1x1 conv (out_w) commutes with the
reassembly, so we conv first on the 4096-pixel grid (v = x @ wo), then
reassemble v directly into the final output: 4x fewer conv FLOPs.

Reassembly mapping (one 105-partition matmul per output row per w-half):
the contraction dim packs 3 v-rows x 35 w-columns of vb = v + bo, where
columns/rows outside the image hold bo itself -- softmax weights sum to
1, so reassembling vb with bo padding yields reassemble(v) + bo exactly,
borders included, and the out-conv bias needs no separate add. Two
overlapping w-half tiles cover the 3x3 w-taps without cross-tile halo.
The banded stationary M [112, 256] is built per row by one gpsimd
local_scatter from a replicated-kt tile (ktrep) that 36 small
partition-shifted SBUF DMAs assemble from the softmaxed kt; v3 tiles
are assembled from vA by ~26 batched partition-remapping SBUF DMAs.

Emission order runs both samples' front-ends before either reassembly so
the PE never stalls on the softmax/scatter dependency chain.
"""
import os
import sys
import numpy as np

sys.path.insert(0, "/opt/trn_rl_repo")

import concourse.bass as bass
import concourse.mybir as mybir
import concourse.tile as tile
from concourse.bass_utils import run_bass_kernel_spmd

F32 = mybir.dt.float32
F16 = mybir.dt.float16
I16 = mybir.dt.int16

U, K, C, H, W = 2, 3, 256, 64, 64
HW = H * W                      # 4096
C4 = C // 4                     # 64
NK, NU, NCH = 9, 4, 36
NCORES = 8
BPC = 2                         # samples per core

PH = 32                         # v3 piece rows (2 pieces per sample)
NQ = 105                        # live v3 partitions: 3 slots x 35 w (incl bo pad)
NIDX = 24                       # scatter idxs per partition: 2 halves x 3 j x 4 u

_cache = {}
last_result = None


# ---------------------------------------------------------------------------
# host-side constant tables
# ---------------------------------------------------------------------------

def _build_idx_tables():
    """[112, 24] int16 scatter indices (single variant).

    Partition q = s*35 + wt (s = v-row slot = tap i, wt = tile w col;
    wt column holds v value at w_v = wt + WOFF, WOFF = -1 / +30, with
    bo padding outside the image so softmax weight-sum-1 supplies the
    out-conv bias). Entry jj = half*12 + j*4 + u scatters ktrep value
    kt[h, w, 3s+j, u] (w = wt-j / wt+31-j) to M col
    half*128 + 64*(u//2) + 2*d + u%2, d = w - 32*half; -1 when the out
    pixel w falls outside the half.
    """
    idx = np.full((112, NIDX), -1, np.int16)
    for q in range(NQ):
        s, wt = q // 35, q % 35
        for half in (0, 1):
            for j in range(3):
                w = wt - j if half == 0 else wt + 31 - j
                if not 32 * half <= w < 32 * half + 32:
                    continue
                d = w - 32 * half
                for u in range(NU):
                    idx[q, half * 12 + j * 4 + u] = \
                        half * 128 + 64 * (u // 2) + 2 * d + (u % 2)
    # pair-batched scatter: one gpsimd scatter fills M for rows (h, h+1)
    # into a [112, 512] tile; odd-row entries target the +256 half
    return np.concatenate([idx, np.where(idx < 0, idx, idx + 256)], axis=1)


# ---------------------------------------------------------------------------
# device program
# ---------------------------------------------------------------------------

def _build_program():
    nc = bass.Bass()

    x2 = nc.declare_dram_parameter("x2", [BPC, HW, C], F16, isOutput=False)
    wd = nc.declare_dram_parameter("wd", [128, 128], F16, isOutput=False)
    bd = nc.declare_dram_parameter("bd", [C4, 1], F32, isOutput=False)
    weA = nc.declare_dram_parameter("weA", [128, 108], F16, isOutput=False)
    weB = nc.declare_dram_parameter("weB", [C4, 108], F16, isOutput=False)
    be = nc.declare_dram_parameter("be", [NCH, 1], F32, isOutput=False)
    wo = nc.declare_dram_parameter("wo", [128, 512], F16, isOutput=False)
    bo16 = nc.declare_dram_parameter("bo16", [128, C], F16, isOutput=False)
    bofill = nc.declare_dram_parameter("bofill", [35, 16 * 2 * C], F16,
                                       isOutput=False)
    out = nc.declare_dram_parameter("out", [BPC, 4 * HW, C], F16, True)

    idx_dram = nc.inline_tensor(_build_idx_tables(), name="idxtab")
    ident_dram = nc.inline_tensor(np.eye(128, dtype=np.float32), name="ident")
    # single-diagonal const: E[w, c] = 1 iff c == w + 72; column slices of
    # it give every banded shift stationary the PE ktrep build needs
    # two 128-row diagonal consts (diag at c = p%64 + 72), one per w-half:
    # L keeps rows with p%64 < 32, H keeps p%64 >= 32. Slicing 64 rows at
    # base e*64 then restricts the contraction to one (e, hf) w-quarter.
    p = np.arange(128)
    ediag = np.zeros((2, 128, 216), np.float16)
    ediag[(p % 64) // 32, p, (p % 64) + 72] = 1.0
    ediag_dram = nc.inline_tensor(
        np.concatenate([ediag[0], ediag[1]], axis=1), name="ediag")

    with tile.TileContext(nc) as tc:
        _emit(tc, nc, x2, wd, bd, weA, weB, be, wo, bo16, bofill, out,
              idx_dram, ident_dram, ediag_dram)
    # raw-Bass path skips Bacc's extended-inst codegen; without this the
    # NEFF compiler sees empty .instr bytes -> "ISA wrong length"
    from concourse.library_overlay import lower_extended_insts
    lower_extended_insts(nc)
    _split_excess_waits(nc)
    return nc


def _split_excess_waits(nc, cap=1):
    """Each TPB instruction has a single EVENTS wait slot; walrus rejects
    multi-wait instructions ("Too many sync wait commands"). Move excess
    waits onto same-engine NoOps immediately before the instruction —
    semantically identical since the engine blocks at the same PC."""
    nid = [0]
    for f in nc.m.functions:
        for b in f.blocks:
            insts = b.instructions
            i = 0
            while i < len(insts):
                ins = insts[i]
                si = getattr(ins, 'sync_info', None)
                if si is not None and si.on_wait and len(si.on_wait) > cap:
                    waits = list(si.on_wait)
                    for w in waits[:-cap]:
                        nop = mybir.InstNoOp(name=f"nopw-{nid[0]}", ins=[],
                                             outs=[])
                        nid[0] += 1
                        nop.engine = ins.engine
                        nop.sync_info = mybir.SyncInfo(on_wait=[w],
                                                       on_update=[])
                        insts.insert(i, nop)
                        i += 1
                    ins.sync_info = mybir.SyncInfo(
                        on_wait=waits[-cap:],
                        on_update=list(si.on_update or []))
                i += 1


def _emit(tc, nc, x2, wd, bd, weA, weB, be, wo, bo16, bofill, out,
          idx_dram, ident_dram, ediag_dram):
    from contextlib import ExitStack
    ablate = set(os.environ.get("KABLATE", "").split(","))
    ctx = ExitStack()
    with ctx:
        consts = ctx.enter_context(tc.tile_pool(name="consts", bufs=1))
        xt_pool = ctx.enter_context(tc.tile_pool(name="xt", bufs=1))
        o1_pool = ctx.enter_context(tc.tile_pool(name="o1", bufs=1))
        enc_pool = ctx.enter_context(tc.tile_pool(name="enc", bufs=1))
        kt_pool = ctx.enter_context(tc.tile_pool(name="kt", bufs=1))
        kr_pool = ctx.enter_context(tc.tile_pool(name="kr", bufs=2))
        v_pool = ctx.enter_context(tc.tile_pool(name="v", bufs=2))
        v3_pool = ctx.enter_context(tc.tile_pool(name="v3", bufs=3))
        mkt_pool = ctx.enter_context(tc.tile_pool(name="mkt", bufs=4))
        out_pool = ctx.enter_context(tc.tile_pool(name="ob", bufs=6))
        ps_dek = ctx.enter_context(tc.tile_pool(name="psdek", bufs=2, space="PSUM"))
        ps_v = ctx.enter_context(tc.tile_pool(name="psv", bufs=2, space="PSUM"))
        ps_o = ctx.enter_context(tc.tile_pool(name="pso", bufs=4, space="PSUM"))

        # ---- constants to SBUF ----
        wd_sb = consts.tile([128, 128], F16)
        nc.sync.dma_start(wd_sb[:], wd[:])
        bd_sb = consts.tile([C4, 1], F32)
        nc.sync.dma_start(bd_sb[:], bd[:])
        weA_sb = consts.tile([128, 108], F16)
        nc.sync.dma_start(weA_sb[:], weA[:])
        weB_sb = consts.tile([C4, 108], F16)
        nc.sync.dma_start(weB_sb[:], weB[:])
        be_sb = consts.tile([NCH, 1], F32)
        nc.sync.dma_start(be_sb[:], be[:])
        wo_sb = consts.tile([128, 512], F16)
        nc.sync.dma_start(wo_sb[:], wo[:])
        idx_sb = consts.tile([112, 2 * NIDX], I16)
        nc.sync.dma_start(idx_sb[:], idx_dram[:])
        ediag_sb = consts.tile([128, 432], F16)
        nc.sync.dma_start(ediag_sb[:], ediag_dram[:])
        bo16_sb = consts.tile([128, C], F16)
        nc.sync.dma_start(bo16_sb[:], bo16[:])
        id_sb = consts.tile([128, 128], F32)
        nc.sync.dma_start(id_sb[:], ident_dram[:])

        from concourse import library_config
        nc.gpsimd.load_library(library_config.local_scatter)

        nrep = int(os.environ.get("KREPEAT", "1"))
        xts, vAs, v3s, kreps = {}, {}, {}, {}

        def front(s):
            # ---- xT [256 c, 4096 pos] via DMA XBAR transpose from DRAM ----
            xt0 = xt_pool.tile([128, HW], F16, tag="xt0")
            xt1 = xt_pool.tile([128, HW], F16, tag="xt1")
            xts[s] = (xt0, xt1)
            # NOTE: both on ONE engine: concurrent XBAR transposes from two
            # engines garble each other (shared xbar state); same-engine
            # back-to-back is safe. Keep un-chunked: a row-sliced src
            # mis-lowers (emitted src pattern spans the full tensor).
            nc.sync.dma_start_transpose(xt0[:], x2[s, :, 0:128])
            nc.sync.dma_start_transpose(xt1[:], x2[s, :, 128:256])

            # ---- out1d: padded + row-shift-duplicated down-conv output ----
            o1 = o1_pool.tile([128, 66 * 66], F16, tag="o1")
            o1v = o1[:].rearrange("p (r c) -> p r c", r=66)
            nc.vector.memset(o1v[:, 0:1, :], 0.0)
            nc.vector.memset(o1v[:, 65:66, :], 0.0)
            nc.vector.memset(o1v[:, :, 0:1], 0.0)
            nc.vector.memset(o1v[:, :, 65:66], 0.0)

            for n in range(8):          # 8 h-rows per 512-chunk
                pd = ps_dek.tile([C4, 512], F32, tag="psdek")
                nc.tensor.matmul(pd[:], wd_sb[:, 0:64],
                                 xt0[:, n * 512:(n + 1) * 512],
                                 start=True, stop=False)
                nc.tensor.matmul(pd[:], wd_sb[:, 64:128],
                                 xt1[:, n * 512:(n + 1) * 512],
                                 start=False, stop=True)
                pdv = pd[:].rearrange("p (r c) -> p r c", r=8)
                nc.vector.tensor_scalar_add(
                    o1v[0:64, 1 + n * 8:9 + n * 8, 1:65], pdv, bd_sb[:])
                nc.vector.tensor_scalar_add(
                    o1v[64:128, n * 8:8 + n * 8, 1:65], pdv, bd_sb[:])

            # ---- enc conv -> enc_out [36, 4096] ----
            enc_sb = enc_pool.tile([NCH, HW], F32, tag="enc")
            for n in range(8):
                pe = ps_dek.tile([NCH, 512], F32, tag="psdek")
                first = True
                for dj in range(3):
                    nc.tensor.matmul(
                        pe[:], weA_sb[:, dj * 36:(dj + 1) * 36],
                        o1v[:, n * 8:n * 8 + 8, dj:dj + 64],
                        start=first, stop=False)
                    first = False
                    nc.tensor.matmul(
                        pe[:], weB_sb[:, dj * 36:(dj + 1) * 36],
                        o1v[0:64, n * 8 + 2:n * 8 + 10, dj:dj + 64],
                        start=False, stop=(dj == 2))
                nc.vector.tensor_scalar_add(
                    enc_sb[:, n * 512:(n + 1) * 512], pe[:], be_sb[:])

            # ---- kt transpose: [4096 pos, 36] chunks + softmax ----
            kts = kt_pool.tile([128, 32 * NCH], F32, tag="kts")
            for c in range(32):
                pt = ps_dek.tile([128, NCH], F32, tag="psdek")
                nc.tensor.matmul(pt[:], enc_sb[:, c * 128:(c + 1) * 128],
                                 id_sb[0:NCH, 0:NCH], is_transpose=True)
                nc.vector.tensor_copy(kts[:, c * NCH:(c + 1) * NCH], pt[:])

            ea = kt_pool.tile([128, 32 * NCH], F32, tag="ea")
            nc.scalar.activation(ea[:], kts[:],
                                 mybir.ActivationFunctionType.Exp)
            sums = kt_pool.tile([128, 128], F32, tag="sums")
            nc.vector.reduce_sum(sums[:].rearrange("p (c u) -> p c u", u=NU),
                                 ea[:].rearrange("p (c k u) -> p c u k",
                                                 k=NK, u=NU),
                                 axis=mybir.AxisListType.X)
            rec = kt_pool.tile([128, 128], F32, tag="rec")
            nc.vector.reciprocal(rec[:], sums[:])
            # kt16 [128 pos%128, 32 chunks x (k,u)] fp16 softmaxed weights
            kt16 = kt_pool.tile([128, 32 * NCH], F16, tag="kt16")
            recb = rec[:].rearrange("p (c u) -> p c u", u=NU) \
                         .unsqueeze(2).broadcast_to([128, 32, NK, NU])
            nc.vector.tensor_tensor(
                kt16[:].rearrange("p (c k u) -> p c k u", k=NK, u=NU),
                ea[:].rearrange("p (c k u) -> p c k u", k=NK, u=NU),
                recb, mybir.AluOpType.mult)

            # ---- ktrep [112, 64 h x 24]: per-partition scatter source ----
            # ktrep[s*35+wt, h*24 + hf*12 + j*4 + u] = kt[h, w, 3s+j, u],
            # w = wt - j (hf=0) / wt + 31 - j (hf=1)
            krep = kr_pool.tile([112, 64 * NIDX], F16, tag="krep")
            kreps[s] = krep
            kv = kt16[:].rearrange("(e w) (hp k u) -> e w hp k u",
                                   e=2, k=NK, u=NU)
            dv = krep[:].rearrange("q (hp e hf j u) -> q hp e hf j u",
                                   hp=32, e=2, hf=2, j=3)
            for sl in range(3):
                for hf in range(2):
                    for j in range(3):
                        off = -j if hf == 0 else 31 - j
                        wlo = j if hf == 0 else j + 1
                        kk = 3 * sl + j
                        for e in range(2):
                            eng = (nc.gpsimd, nc.scalar,
                                   nc.sync)[(6 * sl + 2 * hf + j + e) % 3]
                            eng.dma_start(
                                dv[sl * 35 + wlo:sl * 35 + wlo + 32,
                                   :, e, hf, j, :],
                                kv[e, wlo + off:wlo + off + 32, :, kk, :])

            # ---- v = x @ wo (bias via reassembly bias row) ----
            vA = v_pool.tile([128, 32 * C], F16, tag="vA")
            vAs[s] = vA
            for t in range(32):
                pv = ps_v.tile([128, C], F32, tag="psv")
                nc.tensor.matmul(pv[:], xt0[:, t * 128:(t + 1) * 128],
                                 wo_sb[:, 0:256], start=True, stop=False)
                nc.tensor.matmul(pv[:], xt1[:, t * 128:(t + 1) * 128],
                                 wo_sb[:, 256:512], start=False, stop=True)
                nc.vector.tensor_tensor(vA[:, t * C:(t + 1) * C], pv[:],
                                        bo16_sb[:], mybir.AluOpType.add)

            # ---- v3 pieces: [105, 2 b x 2 halves x 16 a x 256 c] fp16 ----
            # partition s*35+wt = vb[h-1+s, wt+WOFF], vb = v + bo with bo
            # padding outside the image; h = hbase + 2a + b. The (b,hf,a,c)
            # free order makes each (sl,hf,b) repack DMA one contiguous
            # 8KB run per partition on both sides (consecutive vA chunks).
            v3s[s] = []
            vv = vA[:].rearrange("(e w) (t c) -> e w t c", e=2, c=C)
            bfc = bofill[:].rearrange("p (b a c) -> p b a c", b=2, a=16)
            for piece in range(2):
                hbase = piece * PH
                v3 = v3_pool.tile([NQ, PH * 2 * C], F16, tag="v3")
                v3s[s].append(v3)
                v3r = v3[:].rearrange("(sl w) (b hf a c) -> sl w b hf a c",
                                      sl=3, b=2, hf=2, a=16)
                # bo pad columns: hf0 wt=0 (w_v=-1), hf1 wt=34 (w_v=64)
                v3eng = ((nc.gpsimd, nc.gpsimd) if s == 0
                         else (nc.sync, nc.scalar))
                v3eng[0].dma_start(v3r[:, 0, :, 0, :, :], bfc[0:3])
                v3eng[1].dma_start(v3r[:, 34, :, 1, :, :], bfc[0:3])
                # bo pad rows (r=-1 / r=64)
                if piece == 0:   # h=0: slot0, block (b=0, hf, a=0)
                    v3eng[0].dma_start(v3r[0, :, 0, :, 0, :],
                                       bfc[0:35, :, 0, :])
                else:            # h=63: slot2, block (b=1, hf, a=15)
                    v3eng[1].dma_start(v3r[2, :, 1, :, 15, :],
                                       bfc[0:35, :, 0, :])
                for sl in range(3):
                    for hf in range(2):
                        # tile w cols wt (w_v = wt + WOFF in 0..63)
                        wlo = 1 if hf == 0 else 0
                        wsrc = 0 if hf == 0 else 30
                        for b in range(2):
                            # h = hbase + 2a + b, v row r = h - 1 + sl
                            e_src = (b + sl + 1) % 2
                            alo, ahi = 0, 16
                            r0 = hbase + b + sl - 1          # r at a=0
                            if r0 < 0:
                                alo = 1                      # h=0: bo row
                            if r0 + 30 > 63:
                                ahi = 15                     # h=63: bo row
                            t0 = (r0 + 2 * alo) // 2
                            v3eng[(sl + b) % 2].dma_start(
                                v3r[sl, wlo:wlo + 34, b, hf, alo:ahi, :],
                                vv[e_src, wsrc:wsrc + 34,
                                   t0:t0 + ahi - alo, :])

        mkts = {}

        def reasm_row(s, h, r):
            if "reasm" in ablate:
                return
            krep = kreps[s]
            piece, hb = h // PH, h % PH
            if True:
                mkt = mkt_pool.tile([112, 256], F16, tag="mkt")
                mkts[s] = mkt
                if "scatter" not in ablate:
                    nc.gpsimd.local_scatter(
                        mkt[:],
                        krep[:, h * NIDX:(h + 1) * NIDX],
                        idx_sb[:, 0:NIDX],
                        channels=112, num_elems=256, num_idxs=NIDX)
                else:
                    nc.gpsimd.memset(mkt[0:1, 0:2], 0.0)
            mkt = mkts[s]
            moff = 0
            po = ps_o.tile([128, 512], F32, tag="pso")
            v3 = v3s[s][piece]
            a, b = hb // 2, hb % 2
            for hf in range(2):
                blk = (b * 2 + hf) * 16 + a
                nc.tensor.matmul(
                    po[:, hf * C:(hf + 1) * C],
                    mkt[0:NQ, moff + hf * 128:moff + (hf + 1) * 128],
                    v3[0:NQ, blk * C:(blk + 1) * C],
                    start=True, stop=True)
            ob = out_pool.tile([128, 512], F16, tag="ob")
            if r % 2 == 0:
                nc.vector.tensor_copy(ob[:], po[:])
            else:
                nc.scalar.activation(ob[:], po[:],
                                     mybir.ActivationFunctionType.Copy)
            # out row (2h+uh)*128 + 64*hf + p', p' = 2d+uw = partition%64
            engs = ((nc.sync, nc.scalar) if r % 2 == 0
                    else (nc.sync, nc.gpsimd))
            for uh in range(2):
                dst = out[s, (2 * h + uh) * 128:(2 * h + uh) * 128 + 128,
                          :].rearrange("(hf p) c -> p hf c", hf=2)
                engs[uh].dma_start(
                    dst,
                    ob[64 * uh:64 * uh + 64, :]
                    .rearrange("p (hf c) -> p hf c", hf=2))

        # s0 leads s1 by LAG rows so the v3 pool rotation (s1's piece
        # reusing s0's slot) has drained before s1's rows need it.
        LAG = 16
        for _ in range(nrep):
            for s in range(BPC):
                front(s)
            rows = [(0, h) for h in range(LAG)]
            for i in range(H - LAG):
                rows.append((0, LAG + i))
                rows.append((1, i))
            rows += [(1, h) for h in range(H - LAG, H)]
            for r, (s, h) in enumerate(rows):
                reasm_row(s, h, r)


# ---------------------------------------------------------------------------
# host entry
# ---------------------------------------------------------------------------

def _pack_weights(down_w, down_b, enc_w, enc_b, out_w, out_b):
    wd = np.zeros((128, 128), np.float32)
    wdT = down_w[:, :, 0, 0].T.astype(np.float32)       # [256 c, 64]
    wd[:, 0:64] = wdT[0:128]
    wd[:, 64:128] = wdT[128:256]
    weA = np.zeros((128, 108), np.float32)
    weB = np.zeros((C4, 108), np.float32)
    for dj in range(3):
        weA[0:64, dj * 36:(dj + 1) * 36] = enc_w[:, :, 0, dj].T
        weA[64:128, dj * 36:(dj + 1) * 36] = enc_w[:, :, 1, dj].T
        weB[:, dj * 36:(dj + 1) * 36] = enc_w[:, :, 2, dj].T
    woT = out_w[:, :, 0, 0].T.astype(np.float32)        # [256 c, 256 cout]
    wo = np.zeros((128, 512), np.float32)
    wo[:, 0:256] = woT[0:128]
    wo[:, 256:512] = woT[128:256]
    bo16 = np.broadcast_to(out_b.reshape(1, C), (128, C)).astype(np.float16)
    bofill = np.tile(out_b.astype(np.float16), (35, 32)).reshape(35, 16 * 2 * C)
    return {
        "wd": wd.astype(np.float16),
        "bd": down_b.reshape(C4, 1).astype(np.float32),
        "weA": weA.astype(np.float16), "weB": weB.astype(np.float16),
        "be": enc_b.reshape(NCH, 1).astype(np.float32),
        "wo": wo.astype(np.float16),
        "bo16": np.ascontiguousarray(bo16),
        "bofill": np.ascontiguousarray(bofill),
    }


def kernel(x, down_w, down_b, enc_w, enc_b, out_w, out_b):
    global last_result
    if "nc" not in _cache:
        _cache["nc"] = _build_program()
    nc = _cache["nc"]

    x = np.ascontiguousarray(np.asarray(x, np.float32).astype(np.float16))
    shared = _pack_weights(np.asarray(down_w), np.asarray(down_b),
                           np.asarray(enc_w), np.asarray(enc_b),
                           np.asarray(out_w), np.asarray(out_b))
    in_maps = []
    for i in range(NCORES):
        m = dict(shared)
        m["x2"] = np.ascontiguousarray(x[BPC * i:BPC * (i + 1)])
        in_maps.append(m)

    res = run_bass_kernel_spmd(nc, in_maps, core_ids=list(range(NCORES)),
                               trace=bool(os.environ.get("KTRACE")))
    last_result = res
    return np.concatenate([r["out"] for r in res.results],
                          axis=0).astype(np.float32)



# revision 58
# speedup vs baseline: 1.1043x; 1.0028x over previous
"""CARAFE forward as a Bass/Tile kernel for 8 Trainium2 NeuronCores.

Problem (per sample, B=16 total, data-parallel 2 samples/core):
  x [4096, 256] -> down 1x1 conv (256->64) -> enc 3x3 conv (64->36)
  -> pixel_shuffle + softmax over 9 patch weights per upsampled pixel
  -> content-adaptive reassembly of out-conv features -> out [16384, 256]

Key algebraic fusion: the trailing 1x1 conv (out_w) commutes with the
reassembly, so we conv first on the 4096-pixel grid (v = x @ wo), then
reassemble v directly into the final output: 4x fewer conv FLOPs.

Reassembly mapping (one 105-partition matmul per output row per w-half):
the contraction dim packs 3 v-rows x 35 w-columns of vb = v + bo, where
columns/rows outside the image hold bo itself -- softmax weights sum to
1, so reassembling vb with bo padding yields reassemble(v) + bo exactly,
borders included, and the out-conv bias needs no separate add. Two
overlapping w-half tiles cover the 3x3 w-taps without cross-tile halo.
The banded stationary M [112, 256] is built per row by one gpsimd
local_scatter from a replicated-kt tile (ktrep) that 36 small
partition-shifted SBUF DMAs assemble from the softmaxed kt; v3 tiles
are assembled from vA by ~26 batched partition-remapping SBUF DMAs.

Emission order runs both samples' front-ends before either reassembly so
the PE never stalls on the softmax/scatter dependency chain.
"""
import os
import sys
import numpy as np

sys.path.insert(0, "/opt/trn_rl_repo")

import concourse.bass as bass
import concourse.mybir as mybir
import concourse.tile as tile
from concourse.bass_utils import run_bass_kernel_spmd

F32 = mybir.dt.float32
F16 = mybir.dt.float16
I16 = mybir.dt.int16

U, K, C, H, W = 2, 3, 256, 64, 64
HW = H * W                      # 4096
C4 = C // 4                     # 64
NK, NU, NCH = 9, 4, 36
NCORES = 8
BPC = 2                         # samples per core

PH = 32                         # v3 piece rows (2 pieces per sample)
NQ = 105                        # live v3 partitions: 3 slots x 35 w (incl bo pad)
NIDX = 24                       # scatter idxs per partition: 2 halves x 3 j x 4 u

_cache = {}
last_result = None


# ---------------------------------------------------------------------------
# host-side constant tables
# ---------------------------------------------------------------------------

def _build_idx_tables():
    """[112, 24] int16 scatter indices (single variant).

    Partition q = s*35 + wt (s = v-row slot = tap i, wt = tile w col;
    wt column holds v value at w_v = wt + WOFF, WOFF = -1 / +30, with
    bo padding outside the image so softmax weight-sum-1 supplies the
    out-conv bias). Entry jj = half*12 + j*4 + u scatters ktrep value
    kt[h, w, 3s+j, u] (w = wt-j / wt+31-j) to M col
    half*128 + 64*(u//2) + 2*d + u%2, d = w - 32*half; -1 when the out
    pixel w falls outside the half.
    """
    idx = np.full((112, NIDX), -1, np.int16)
    for q in range(NQ):
        s, wt = q // 35, q % 35
        for half in (0, 1):
            for j in range(3):
                w = wt - j if half == 0 else wt + 31 - j
                if not 32 * half <= w < 32 * half + 32:
                    continue
                d = w - 32 * half
                for u in range(NU):
                    idx[q, half * 12 + j * 4 + u] = \
                        half * 128 + 64 * (u // 2) + 2 * d + (u % 2)
    # pair-batched scatter: one gpsimd scatter fills M for rows (h, h+1)
    # into a [112, 512] tile; odd-row entries target the +256 half
    return np.concatenate([idx, np.where(idx < 0, idx, idx + 256)], axis=1)


# ---------------------------------------------------------------------------
# device program
# ---------------------------------------------------------------------------

def _build_program():
    nc = bass.Bass()

    x2 = nc.declare_dram_parameter("x2", [BPC, HW, C], F16, isOutput=False)
    wd = nc.declare_dram_parameter("wd", [128, 128], F16, isOutput=False)
    bd = nc.declare_dram_parameter("bd", [C4, 1], F32, isOutput=False)
    weA = nc.declare_dram_parameter("weA", [128, 108], F16, isOutput=False)
    weB = nc.declare_dram_parameter("weB", [C4, 108], F16, isOutput=False)
    be = nc.declare_dram_parameter("be", [NCH, 1], F32, isOutput=False)
    wo = nc.declare_dram_parameter("wo", [128, 512], F16, isOutput=False)
    bo16 = nc.declare_dram_parameter("bo16", [128, C], F16, isOutput=False)
    bofill = nc.declare_dram_parameter("bofill", [35, 16 * 2 * C], F16,
                                       isOutput=False)
    out = nc.declare_dram_parameter("out", [BPC, 4 * HW, C], F16, True)

    idx_dram = nc.inline_tensor(_build_idx_tables(), name="idxtab")
    ident_dram = nc.inline_tensor(np.eye(128, dtype=np.float32), name="ident")
    # single-diagonal const: E[w, c] = 1 iff c == w + 72; column slices of
    # it give every banded shift stationary the PE ktrep build needs
    # two 128-row diagonal consts (diag at c = p%64 + 72), one per w-half:
    # L keeps rows with p%64 < 32, H keeps p%64 >= 32. Slicing 64 rows at
    # base e*64 then restricts the contraction to one (e, hf) w-quarter.
    p = np.arange(128)
    ediag = np.zeros((2, 128, 216), np.float16)
    ediag[(p % 64) // 32, p, (p % 64) + 72] = 1.0
    ediag_dram = nc.inline_tensor(
        np.concatenate([ediag[0], ediag[1]], axis=1), name="ediag")

    with tile.TileContext(nc) as tc:
        _emit(tc, nc, x2, wd, bd, weA, weB, be, wo, bo16, bofill, out,
              idx_dram, ident_dram, ediag_dram)
    # raw-Bass path skips Bacc's extended-inst codegen; without this the
    # NEFF compiler sees empty .instr bytes -> "ISA wrong length"
    from concourse.library_overlay import lower_extended_insts
    lower_extended_insts(nc)
    _split_excess_waits(nc)
    return nc


def _split_excess_waits(nc, cap=1):
    """Each TPB instruction has a single EVENTS wait slot; walrus rejects
    multi-wait instructions ("Too many sync wait commands"). Move excess
    waits onto same-engine NoOps immediately before the instruction —
    semantically identical since the engine blocks at the same PC."""
    nid = [0]
    for f in nc.m.functions:
        for b in f.blocks:
            insts = b.instructions
            i = 0
            while i < len(insts):
                ins = insts[i]
                si = getattr(ins, 'sync_info', None)
                if si is not None and si.on_wait and len(si.on_wait) > cap:
                    waits = list(si.on_wait)
                    for w in waits[:-cap]:
                        nop = mybir.InstNoOp(name=f"nopw-{nid[0]}", ins=[],
                                             outs=[])
                        nid[0] += 1
                        nop.engine = ins.engine
                        nop.sync_info = mybir.SyncInfo(on_wait=[w],
                                                       on_update=[])
                        insts.insert(i, nop)
                        i += 1
                    ins.sync_info = mybir.SyncInfo(
                        on_wait=waits[-cap:],
                        on_update=list(si.on_update or []))
                i += 1


def _emit(tc, nc, x2, wd, bd, weA, weB, be, wo, bo16, bofill, out,
          idx_dram, ident_dram, ediag_dram):
    from contextlib import ExitStack
    ablate = set(os.environ.get("KABLATE", "").split(","))
    ctx = ExitStack()
    with ctx:
        consts = ctx.enter_context(tc.tile_pool(name="consts", bufs=1))
        xt_pool = ctx.enter_context(tc.tile_pool(name="xt", bufs=1))
        o1_pool = ctx.enter_context(tc.tile_pool(name="o1", bufs=1))
        enc_pool = ctx.enter_context(tc.tile_pool(name="enc", bufs=1))
        kt_pool = ctx.enter_context(tc.tile_pool(name="kt", bufs=1))
        kr_pool = ctx.enter_context(tc.tile_pool(name="kr", bufs=2))
        v_pool = ctx.enter_context(tc.tile_pool(name="v", bufs=2))
        v3_pool = ctx.enter_context(tc.tile_pool(name="v3", bufs=3))
        mkt_pool = ctx.enter_context(tc.tile_pool(name="mkt", bufs=4))
        out_pool = ctx.enter_context(tc.tile_pool(name="ob", bufs=6))
        ps_dek = ctx.enter_context(tc.tile_pool(name="psdek", bufs=2, space="PSUM"))
        ps_v = ctx.enter_context(tc.tile_pool(name="psv", bufs=2, space="PSUM"))
        ps_o = ctx.enter_context(tc.tile_pool(name="pso", bufs=4, space="PSUM"))

        # ---- constants to SBUF ----
        wd_sb = consts.tile([128, 128], F16)
        nc.sync.dma_start(wd_sb[:], wd[:])
        bd_sb = consts.tile([C4, 1], F32)
        nc.sync.dma_start(bd_sb[:], bd[:])
        weA_sb = consts.tile([128, 108], F16)
        nc.sync.dma_start(weA_sb[:], weA[:])
        weB_sb = consts.tile([C4, 108], F16)
        nc.sync.dma_start(weB_sb[:], weB[:])
        be_sb = consts.tile([NCH, 1], F32)
        nc.sync.dma_start(be_sb[:], be[:])
        wo_sb = consts.tile([128, 512], F16)
        nc.sync.dma_start(wo_sb[:], wo[:])
        idx_sb = consts.tile([112, 2 * NIDX], I16)
        nc.sync.dma_start(idx_sb[:], idx_dram[:])
        ediag_sb = consts.tile([128, 432], F16)
        nc.sync.dma_start(ediag_sb[:], ediag_dram[:])
        bo16_sb = consts.tile([128, C], F16)
        nc.sync.dma_start(bo16_sb[:], bo16[:])
        id_sb = consts.tile([128, 128], F32)
        nc.sync.dma_start(id_sb[:], ident_dram[:])

        from concourse import library_config
        nc.gpsimd.load_library(library_config.local_scatter)

        nrep = int(os.environ.get("KREPEAT", "1"))
        xts, vAs, v3s, kreps = {}, {}, {}, {}

        def front(s):
            # ---- xT [256 c, 4096 pos] via DMA XBAR transpose from DRAM ----
            xt0 = xt_pool.tile([128, HW], F16, tag="xt0")
            xt1 = xt_pool.tile([128, HW], F16, tag="xt1")
            xts[s] = (xt0, xt1)
            # NOTE: both on ONE engine: concurrent XBAR transposes from two
            # engines garble each other (shared xbar state); same-engine
            # back-to-back is safe. Keep un-chunked: a row-sliced src
            # mis-lowers (emitted src pattern spans the full tensor).
            nc.sync.dma_start_transpose(xt0[:], x2[s, :, 0:128])
            nc.sync.dma_start_transpose(xt1[:], x2[s, :, 128:256])

            # ---- out1d: padded + row-shift-duplicated down-conv output ----
            o1 = o1_pool.tile([128, 66 * 66], F16, tag="o1")
            o1v = o1[:].rearrange("p (r c) -> p r c", r=66)
            nc.vector.memset(o1v[:, 0:1, :], 0.0)
            nc.vector.memset(o1v[:, 65:66, :], 0.0)
            nc.vector.memset(o1v[:, :, 0:1], 0.0)
            nc.vector.memset(o1v[:, :, 65:66], 0.0)

            for n in range(8):          # 8 h-rows per 512-chunk
                pd = ps_dek.tile([C4, 512], F32, tag="psdek")
                nc.tensor.matmul(pd[:], wd_sb[:, 0:64],
                                 xt0[:, n * 512:(n + 1) * 512],
                                 start=True, stop=False)
                nc.tensor.matmul(pd[:], wd_sb[:, 64:128],
                                 xt1[:, n * 512:(n + 1) * 512],
                                 start=False, stop=True)
                pdv = pd[:].rearrange("p (r c) -> p r c", r=8)
                nc.vector.tensor_scalar_add(
                    o1v[0:64, 1 + n * 8:9 + n * 8, 1:65], pdv, bd_sb[:])
                nc.vector.tensor_scalar_add(
                    o1v[64:128, n * 8:8 + n * 8, 1:65], pdv, bd_sb[:])

            # ---- enc conv -> enc_out [36, 4096] ----
            enc_sb = enc_pool.tile([NCH, HW], F32, tag="enc")
            for n in range(8):
                pe = ps_dek.tile([NCH, 512], F32, tag="psdek")
                first = True
                for dj in range(3):
                    nc.tensor.matmul(
                        pe[:], weA_sb[:, dj * 36:(dj + 1) * 36],
                        o1v[:, n * 8:n * 8 + 8, dj:dj + 64],
                        start=first, stop=False)
                    first = False
                    nc.tensor.matmul(
                        pe[:], weB_sb[:, dj * 36:(dj + 1) * 36],
                        o1v[0:64, n * 8 + 2:n * 8 + 10, dj:dj + 64],
                        start=False, stop=(dj == 2))
                nc.vector.tensor_scalar_add(
                    enc_sb[:, n * 512:(n + 1) * 512], pe[:], be_sb[:])

            # ---- kt transpose: [4096 pos, 36] chunks + softmax ----
            kts = kt_pool.tile([128, 32 * NCH], F32, tag="kts")
            for c in range(32):
                pt = ps_dek.tile([128, NCH], F32, tag="psdek")
                nc.tensor.matmul(pt[:], enc_sb[:, c * 128:(c + 1) * 128],
                                 id_sb[0:NCH, 0:NCH], is_transpose=True)
                nc.vector.tensor_copy(kts[:, c * NCH:(c + 1) * NCH], pt[:])

            ea = kt_pool.tile([128, 32 * NCH], F32, tag="ea")
            nc.scalar.activation(ea[:], kts[:],
                                 mybir.ActivationFunctionType.Exp)
            sums = kt_pool.tile([128, 128], F32, tag="sums")
            nc.vector.reduce_sum(sums[:].rearrange("p (c u) -> p c u", u=NU),
                                 ea[:].rearrange("p (c k u) -> p c u k",
                                                 k=NK, u=NU),
                                 axis=mybir.AxisListType.X)
            rec = kt_pool.tile([128, 128], F32, tag="rec")
            nc.vector.reciprocal(rec[:], sums[:])
            # kt16 [128 pos%128, 32 chunks x (k,u)] fp16 softmaxed weights
            kt16 = kt_pool.tile([128, 32 * NCH], F16, tag="kt16")
            recb = rec[:].rearrange("p (c u) -> p c u", u=NU) \
                         .unsqueeze(2).broadcast_to([128, 32, NK, NU])
            nc.vector.tensor_tensor(
                kt16[:].rearrange("p (c k u) -> p c k u", k=NK, u=NU),
                ea[:].rearrange("p (c k u) -> p c k u", k=NK, u=NU),
                recb, mybir.AluOpType.mult)

            # ---- ktrep [112, 64 h x 24]: per-partition scatter source ----
            # ktrep[s*35+wt, h*24 + hf*12 + j*4 + u] = kt[h, w, 3s+j, u],
            # w = wt - j (hf=0) / wt + 31 - j (hf=1)
            krep = kr_pool.tile([112, 64 * NIDX], F16, tag="krep")
            kreps[s] = krep
            kv = kt16[:].rearrange("(e w) (hp k u) -> e w hp k u",
                                   e=2, k=NK, u=NU)
            dv = krep[:].rearrange("q (hp e hf j u) -> q hp e hf j u",
                                   hp=32, e=2, hf=2, j=3)
            for sl in range(3):
                for hf in range(2):
                    for j in range(3):
                        off = -j if hf == 0 else 31 - j
                        wlo = j if hf == 0 else j + 1
                        kk = 3 * sl + j
                        for e in range(2):
                            eng = (nc.gpsimd, nc.scalar,
                                   nc.sync)[(6 * sl + 2 * hf + j + e) % 3]
                            eng.dma_start(
                                dv[sl * 35 + wlo:sl * 35 + wlo + 32,
                                   :, e, hf, j, :],
                                kv[e, wlo + off:wlo + off + 32, :, kk, :])

            # ---- v = x @ wo (bias via reassembly bias row) ----
            vA = v_pool.tile([128, 32 * C], F16, tag="vA")
            vAs[s] = vA
            for t in range(32):
                pv = ps_v.tile([128, C], F32, tag="psv")
                nc.tensor.matmul(pv[:], xt0[:, t * 128:(t + 1) * 128],
                                 wo_sb[:, 0:256], start=True, stop=False)
                nc.tensor.matmul(pv[:], xt1[:, t * 128:(t + 1) * 128],
                                 wo_sb[:, 256:512], start=False, stop=True)
                nc.vector.tensor_tensor(vA[:, t * C:(t + 1) * C], pv[:],
                                        bo16_sb[:], mybir.AluOpType.add)

            # ---- v3 pieces: [105, 2 b x 2 halves x 16 a x 256 c] fp16 ----
            # partition s*35+wt = vb[h-1+s, wt+WOFF], vb = v + bo with bo
            # padding outside the image; h = hbase + 2a + b. The (b,hf,a,c)
            # free order makes each (sl,hf,b) repack DMA one contiguous
            # 8KB run per partition on both sides (consecutive vA chunks).
            v3s[s] = []
            vv = vA[:].rearrange("(e w) (t c) -> e w t c", e=2, c=C)
            bfc = bofill[:].rearrange("p (b a c) -> p b a c", b=2, a=16)
            for piece in range(2):
                hbase = piece * PH
                v3 = v3_pool.tile([NQ, PH * 2 * C], F16, tag="v3")
                v3s[s].append(v3)
                v3r = v3[:].rearrange("(sl w) (b hf a c) -> sl w b hf a c",
                                      sl=3, b=2, hf=2, a=16)
                # bo pad columns: hf0 wt=0 (w_v=-1), hf1 wt=34 (w_v=64)
                v3eng = ((nc.gpsimd, nc.gpsimd) if s == 0
                         else (nc.sync, nc.scalar))
                v3eng[0].dma_start(v3r[:, 0, :, 0, :, :], bfc[0:3])
                v3eng[1].dma_start(v3r[:, 34, :, 1, :, :], bfc[0:3])
                # bo pad rows (r=-1 / r=64)
                if piece == 0:   # h=0: slot0, block (b=0, hf, a=0)
                    v3eng[0].dma_start(v3r[0, :, 0, :, 0, :],
                                       bfc[0:35, :, 0, :])
                else:            # h=63: slot2, block (b=1, hf, a=15)
                    v3eng[1].dma_start(v3r[2, :, 1, :, 15, :],
                                       bfc[0:35, :, 0, :])
                for sl in range(3):
                    for hf in range(2):
                        # tile w cols wt (w_v = wt + WOFF in 0..63)
                        wlo = 1 if hf == 0 else 0
                        wsrc = 0 if hf == 0 else 30
                        for b in range(2):
                            # h = hbase + 2a + b, v row r = h - 1 + sl
                            e_src = (b + sl + 1) % 2
                            alo, ahi = 0, 16
                            r0 = hbase + b + sl - 1          # r at a=0
                            if r0 < 0:
                                alo = 1                      # h=0: bo row
                            if r0 + 30 > 63:
                                ahi = 15                     # h=63: bo row
                            t0 = (r0 + 2 * alo) // 2
                            v3eng[(sl + b) % 2].dma_start(
                                v3r[sl, wlo:wlo + 34, b, hf, alo:ahi, :],
                                vv[e_src, wsrc:wsrc + 34,
                                   t0:t0 + ahi - alo, :])

        mkts = {}

        def reasm_row(s, h, r):
            if "reasm" in ablate:
                return
            krep = kreps[s]
            piece, hb = h // PH, h % PH
            if h % 2 == 0:
                # one gpsimd scatter builds M for rows (h, h+1)
                mkt = mkt_pool.tile([112, 512], F16, tag="mkt")
                mkts[s] = mkt
                if "scatter" not in ablate:
                    nc.gpsimd.local_scatter(
                        mkt[:],
                        krep[:, h * NIDX:(h + 2) * NIDX],
                        idx_sb[:],
                        channels=112, num_elems=512, num_idxs=2 * NIDX)
                else:
                    nc.gpsimd.memset(mkt[0:1, 0:2], 0.0)
            mkt = mkts[s]
            moff = (h % 2) * 256
            po = ps_o.tile([128, 512], F32, tag="pso")
            v3 = v3s[s][piece]
            a, b = hb // 2, hb % 2
            for hf in range(2):
                blk = (b * 2 + hf) * 16 + a
                nc.tensor.matmul(
                    po[:, hf * C:(hf + 1) * C],
                    mkt[0:NQ, moff + hf * 128:moff + (hf + 1) * 128],
                    v3[0:NQ, blk * C:(blk + 1) * C],
                    start=True, stop=True)
            ob = out_pool.tile([128, 512], F16, tag="ob")
            if r % 2 == 0:
                nc.vector.tensor_copy(ob[:], po[:])
            else:
                nc.scalar.activation(ob[:], po[:],
                                     mybir.ActivationFunctionType.Copy)
            # out row (2h+uh)*128 + 64*hf + p', p' = 2d+uw = partition%64
            engs = ((nc.sync, nc.scalar) if r % 2 == 0
                    else (nc.sync, nc.gpsimd))
            for uh in range(2):
                dst = out[s, (2 * h + uh) * 128:(2 * h + uh) * 128 + 128,
                          :].rearrange("(hf p) c -> p hf c", hf=2)
                engs[uh].dma_start(
                    dst,
                    ob[64 * uh:64 * uh + 64, :]
                    .rearrange("p (hf c) -> p hf c", hf=2))

        # s0 leads s1 by LAG rows so the v3 pool rotation (s1's piece
        # reusing s0's slot) has drained before s1's rows need it.
        LAG = 16
        for _ in range(nrep):
            for s in range(BPC):
                front(s)
            rows = [(0, h) for h in range(LAG)]
            for i in range(H - LAG):
                rows.append((0, LAG + i))
                rows.append((1, i))
            rows += [(1, h) for h in range(H - LAG, H)]
            for r, (s, h) in enumerate(rows):
                reasm_row(s, h, r)


# ---------------------------------------------------------------------------
# host entry
# ---------------------------------------------------------------------------

def _pack_weights(down_w, down_b, enc_w, enc_b, out_w, out_b):
    wd = np.zeros((128, 128), np.float32)
    wdT = down_w[:, :, 0, 0].T.astype(np.float32)       # [256 c, 64]
    wd[:, 0:64] = wdT[0:128]
    wd[:, 64:128] = wdT[128:256]
    weA = np.zeros((128, 108), np.float32)
    weB = np.zeros((C4, 108), np.float32)
    for dj in range(3):
        weA[0:64, dj * 36:(dj + 1) * 36] = enc_w[:, :, 0, dj].T
        weA[64:128, dj * 36:(dj + 1) * 36] = enc_w[:, :, 1, dj].T
        weB[:, dj * 36:(dj + 1) * 36] = enc_w[:, :, 2, dj].T
    woT = out_w[:, :, 0, 0].T.astype(np.float32)        # [256 c, 256 cout]
    wo = np.zeros((128, 512), np.float32)
    wo[:, 0:256] = woT[0:128]
    wo[:, 256:512] = woT[128:256]
    bo16 = np.broadcast_to(out_b.reshape(1, C), (128, C)).astype(np.float16)
    bofill = np.tile(out_b.astype(np.float16), (35, 32)).reshape(35, 16 * 2 * C)
    return {
        "wd": wd.astype(np.float16),
        "bd": down_b.reshape(C4, 1).astype(np.float32),
        "weA": weA.astype(np.float16), "weB": weB.astype(np.float16),
        "be": enc_b.reshape(NCH, 1).astype(np.float32),
        "wo": wo.astype(np.float16),
        "bo16": np.ascontiguousarray(bo16),
        "bofill": np.ascontiguousarray(bofill),
    }


def kernel(x, down_w, down_b, enc_w, enc_b, out_w, out_b):
    global last_result
    if "nc" not in _cache:
        _cache["nc"] = _build_program()
    nc = _cache["nc"]

    x = np.ascontiguousarray(np.asarray(x, np.float32).astype(np.float16))
    shared = _pack_weights(np.asarray(down_w), np.asarray(down_b),
                           np.asarray(enc_w), np.asarray(enc_b),
                           np.asarray(out_w), np.asarray(out_b))
    in_maps = []
    for i in range(NCORES):
        m = dict(shared)
        m["x2"] = np.ascontiguousarray(x[BPC * i:BPC * (i + 1)])
        in_maps.append(m)

    res = run_bass_kernel_spmd(nc, in_maps, core_ids=list(range(NCORES)),
                               trace=bool(os.environ.get("KTRACE")))
    last_result = res
    return np.concatenate([r["out"] for r in res.results],
                          axis=0).astype(np.float32)

